# revision 31
# baseline (speedup 1.0000x reference)
"""AgentAttention Trainium2 kernel: 8-core data-parallel over batch.

Layouts (per core, 4 batches):
  xT      [4, 768, 1176] bf16  (c-major x)
  qkT     c-major q,k: 12 sbuf tiles [128, 1176] (tiles 0-5 = q rows, 6-11 = k rows)
  v_ext   pos-major v with per-head ones column (col 64): 10 tiles [128, 12*65]
  agT     pooled agent tokens (sums over 4x4 blocks), c-major [128, 49] x6
  aoT     c-major attention output (fp32r) [128, 1176] x6 -> proj -> pos-major out
Matmuls bf16 (fp32r for proj), fp32 psum. Softmax scale folded into ACT exp
scale (0.125 stage1; 0.125/16 stages 2/3 -- agent tokens are pooled SUMS).
Biases are added via K=1 matmul accumulation into psum.
PSUM: tag A = [128,1024] (2 banks) x2, tag P = [128,512] (1 bank) x4.
"""

import sys

sys.path.insert(0, "/opt/trn_rl_repo")

import numpy as np
import ml_dtypes

import concourse.bass as bass
import concourse.mybir as mybir
import concourse.tile as tile
from concourse import bacc, bass_utils
from concourse.masks import make_identity

BF = mybir.dt.bfloat16
F32 = mybir.dt.float32
FR = mybir.dt.float32r
AF = mybir.ActivationFunctionType
AX = mybir.AxisListType

N_CORES = 8
B, N, C = 32, 1176, 768
NB = B // N_CORES
H, HD = 12, 64
N_MT, N_S = 392, 784
A = 49
SCALE1 = 0.125
SCALE23 = 0.125 / 16.0

POS_T = [(pt * 128, min(128, N - pt * 128)) for pt in range(10)]
KEY1_T = [(0, 128), (128, 128), (256, 128), (384, 8)]
NCHUNK = [(0, 392), (392, 392), (784, 392)]
TSP = 116  # transpose chunk col spacing (>=113, even)


def build_program():
    nc = bacc.Bacc("TRN2", debug=False, num_devices=N_CORES)

    xT_d = nc.dram_tensor("xT", [NB, C, N], BF, kind="ExternalInput").ap()
    wqkT_d = nc.dram_tensor("wqkT", [C, 3 * C], BF, kind="ExternalInput").ap()
    wpjT_d = nc.dram_tensor("wpjT", [C, C], F32, kind="ExternalInput").ap()
    bqk_d = nc.dram_tensor("bqk", [1, 3 * C], BF, kind="ExternalInput").ap()
    bpj_d = nc.dram_tensor("bpj", [1, C], F32, kind="ExternalInput").ap()
    out_d = nc.dram_tensor("out", [NB, N, C], F32, kind="ExternalOutput").ap()

    with tile.TileContext(nc) as tc, nc.allow_low_precision(reason="attn bf16"):
        with (
            tc.tile_pool(name="const", bufs=1) as cpool,
            tc.tile_pool(name="work", bufs=2) as wpool,
            tc.tile_pool(name="hold", bufs=1) as hpool,
            tc.tile_pool(name="psum", bufs=2, space="PSUM") as ppool,
        ):
            # ---- one-time constants/weights ----
            wq = [
                cpool.tile([128, 3 * C], BF, tag=f"wq{i}", name=f"wq{i}")
                for i in range(6)
            ]
            for i in range(6):
                nc.sync.dma_start(wq[i][:], wqkT_d[128 * i : 128 * (i + 1), :])
            sb_bqk = cpool.tile([1, 3 * C], BF, tag="bqk")
            nc.sync.dma_start(sb_bqk[:], bqk_d[:])
            ones = cpool.tile([1, 512], BF, tag="ones")
            nc.gpsimd.memset(ones[:], 1.0)
            ident = cpool.tile([128, 128], BF, tag="ident")
            make_identity(nc, ident[:])
            # proj weights (needed late) load after qkv weights
            wp = [
                cpool.tile([128, C], FR, tag=f"wp{i}", name=f"wp{i}") for i in range(6)
            ]
            for i in range(6):
                wps = wpool.tile([128, C], F32, tag="wps", name="wps")
                nc.sync.dma_start(wps[:], wpjT_d[128 * i : 128 * (i + 1), :])
                nc.vector.tensor_copy(wp[i][:], wps[:])
            ones_r = cpool.tile([1, 512], FR, tag="ones_r")
            onesf = wpool.tile([1, 512], F32, tag="onesf", name="onesf")
            nc.gpsimd.memset(onesf[:], 1.0)
            nc.vector.tensor_copy(ones_r[:], onesf[:])
            bpj_r = cpool.tile([1, C], FR, tag="bpj_r")
            bpjf = wpool.tile([1, C], F32, tag="bpjf", name="bpjf")
            nc.sync.dma_start(bpjf[:], bpj_d[:])
            nc.vector.tensor_copy(bpj_r[:], bpjf[:])

            for b in range(NB):
                # ---- load xT ----
                xT = [
                    wpool.tile([128, N], BF, tag=f"xT{i}", name=f"xT{i}", bufs=1)
                    for i in range(6)
                ]
                for i in range(6):
                    nc.sync.dma_start(xT[i][:], xT_d[b, 128 * i : 128 * (i + 1), :])

                # ---- phase Q: qkT c-major (q,k rows), bias via K=1 matmul ----
                qkT = [None] * 12
                for m in [0, 6, 1, 7, 2, 8, 3, 9, 4, 10, 5, 11]:
                    ps = ppool.tile([128, 2 * 512], F32, tag="A", name="psA")
                    ps2 = ppool.tile([128, 512], F32, tag="P", name="psP", bufs=4)
                    for j, (n0, nsz) in enumerate(NCHUNK):
                        dst = ps[:, 512 * j : 512 * j + nsz] if j < 2 else ps2[:, 0:nsz]
                        for kt in range(6):
                            nc.tensor.matmul(
                                dst,
                                wq[kt][:, 128 * m : 128 * (m + 1)],
                                xT[kt][:, n0 : n0 + nsz],
                                start=(kt == 0),
                                stop=False,
                            )
                        nc.tensor.matmul(
                            dst,
                            sb_bqk[0:1, 128 * m : 128 * (m + 1)],
                            ones[0:1, 0:nsz],
                            start=False,
                            stop=True,
                        )
                    t = hpool.tile([128, N], BF, tag=f"qkT{m}", name=f"qkT{m}")
                    qkT[m] = t
                    nc.scalar.copy(
                        t[:, 0:784].rearrange("p (c x) -> p c x", c=2),
                        ps[:].rearrange("p (c x) -> p c x", c=2)[:, :, 0:392],
                    )
                    nc.scalar.copy(t[:, 784:1176], ps2[:, 0:392])

                # ---- phase V: pos-major v_ext with ones cols ----
                v_ext = []
                for pt, (p0, psz) in enumerate(POS_T):
                    ps = ppool.tile([128, 2 * 512], F32, tag="A", name="psA")
                    for c0, csz in [(0, 512), (512, 256)]:
                        for kt in range(6):
                            nc.tensor.matmul(
                                ps[0:psz, c0 : c0 + csz],
                                xT[kt][:, p0 : p0 + psz],
                                wq[kt][:, 2 * C + c0 : 2 * C + c0 + csz],
                                start=(kt == 0),
                                stop=False,
                            )
                        nc.tensor.matmul(
                            ps[0:psz, c0 : c0 + csz],
                            ones[0:1, 0:psz],
                            sb_bqk[0:1, 2 * C + c0 : 2 * C + c0 + csz],
                            start=False,
                            stop=True,
                        )
                    vt = hpool.tile([128, H * 65], BF, tag=f"vx{pt}", name=f"vx{pt}")
                    v_ext.append(vt)
                    if b == 0:
                        # bufs=1 slot memory persists across batches; evac only
                        # writes the 64 v columns, so ones survive
                        nc.vector.memset(
                            vt[:].rearrange("p (h e) -> p h e", e=65)[:, :, 64:65], 1.0
                        )
                    nc.scalar.copy(
                        vt[0:psz].rearrange("p (h e) -> p h e", e=65)[:, :, 0:64],
                        ps[0:psz, 0:768].rearrange("p (h d) -> p h d", d=64),
                    )

                # ---- agent pooling: sum 4x4 blocks of q_s -> agT (c-major) ----
                agT = []
                for ct in range(6):
                    t1 = wpool.tile([128, 196], F32, tag="t1")
                    qs = qkT[ct][:, N_MT:N]  # [128, 784], idx = i*28 + aj*4 + dj
                    q4 = qs.rearrange("p (x dj) -> p x dj", dj=4)
                    nc.gpsimd.tensor_add(t1[:, 0:196], q4[:, :, 0:1], q4[:, :, 1:2])
                    nc.gpsimd.tensor_add(t1[:, 0:196], t1[:, 0:196], q4[:, :, 2:3])
                    nc.gpsimd.tensor_add(t1[:, 0:196], t1[:, 0:196], q4[:, :, 3:4])
                    ag = wpool.tile([128, A], BF, tag=f"ag{ct}", name=f"ag{ct}")
                    agT.append(ag)
                    # t1 idx = 28*ai + 7*di + aj -> view (ai, aj, di)
                    t4 = t1[:, 0:196].rearrange("p (ai di aj) -> p ai aj di", ai=7, di=4)
                    t2 = wpool.tile([128, A], F32, tag="t2")
                    nc.gpsimd.tensor_add(t2[:, 0:A], t4[:, :, :, 0:1], t4[:, :, :, 1:2])
                    nc.gpsimd.tensor_add(t2[:, 0:A], t2[:, 0:A], t4[:, :, :, 2:3])
                    nc.gpsimd.tensor_add(ag[:, 0:A], t2[:, 0:A], t4[:, :, :, 3:4])

                # attention output accumulates here (c-major, fp32r)
                aoT = [
                    hpool.tile([128, N], FR, tag=f"ao{i}", name=f"ao{i}")
                    for i in range(6)
                ]

                # ---- stage 1: one head ----
                def stage1(h):
                    qt, qo = h // 2, (h % 2) * 64
                    scA = ppool.tile([128, 2 * 512], F32, tag="A", name="psA")
                    sc2 = ppool.tile([128, 512], F32, tag="P", name="psP", bufs=4)
                    scP = ppool.tile([128, 512], F32, tag="P", name="psP", bufs=4)
                    for j, (k0, ksz) in enumerate(KEY1_T):
                        dst = (
                            scA[0:ksz, 512 * j : 512 * j + 392]
                            if j < 2
                            else (sc2[0:ksz, 0:392] if j == 2 else scP[0:ksz, 0:392])
                        )
                        nc.tensor.matmul(
                            dst,
                            qkT[6 + qt][qo : qo + 64, k0 : k0 + ksz],
                            qkT[qt][qo : qo + 64, 0:N_MT],
                            start=True,
                            stop=True,
                        )
                    e1 = wpool.tile([128, 4 * 392], BF, tag="e1")
                    nc.scalar.activation(
                        e1[:, 0:784].rearrange("p (c x) -> p c x", c=2),
                        scA[:].rearrange("p (c x) -> p c x", c=2)[:, :, 0:392],
                        AF.Exp,
                        scale=SCALE1,
                    )
                    nc.scalar.activation(
                        e1[:, 784:1176], sc2[:, 0:392], AF.Exp, scale=SCALE1
                    )
                    nc.scalar.activation(
                        e1[0:8, 1176:1568], scP[0:8, 0:392], AF.Exp, scale=SCALE1
                    )
                    pv = ppool.tile([128, 512], F32, tag="P", name="psP", bufs=4)
                    for j, (k0, ksz) in enumerate(KEY1_T):
                        nc.tensor.matmul(
                            pv[0:65, 0:392],
                            v_ext[j][0:ksz, 65 * h : 65 * h + 65],
                            e1[0:ksz, 392 * j : 392 * j + 392],
                            start=(j == 0),
                            stop=(j == 3),
                        )
                    se = wpool.tile([1, 392], F32, tag="se")
                    nc.vector.tensor_copy(se[:, 0:392], pv[64:65, 0:392])
                    rc = wpool.tile([1, 392], F32, tag="rc")
                    nc.vector.reciprocal_approx_fast(out=rc[:, 0:392], in_=se[:, 0:392])
                    bc = wpool.tile([64, 392], F32, tag="bc")
                    nc.gpsimd.partition_broadcast(bc[:], rc[0:1, 0:392])
                    nc.vector.tensor_mul(
                        aoT[qt][qo : qo + 64, 0:N_MT], pv[0:64, 0:392], bc[:]
                    )

                # ---- stages 1+2+3 interleaved per head pair ----
                for p2 in range(6):
                    stage1(2 * p2)
                    stage1(2 * p2 + 1)
                    # stage 2 scores (both heads packed at partition 0/64)
                    scA = ppool.tile([128, 2 * 512], F32, tag="A", name="psA")
                    sc2 = ppool.tile([128, 512], F32, tag="P", name="psP", bufs=4)
                    for hp in range(2):
                        h = 2 * p2 + hp
                        qt, qo = h // 2, (h % 2) * 64
                        for j, (n0, nsz) in enumerate(NCHUNK):
                            dst = (
                                scA[64 * hp : 64 * hp + 49, 512 * j : 512 * j + nsz]
                                if j < 2
                                else sc2[64 * hp : 64 * hp + 49, 0:nsz]
                            )
                            nc.tensor.matmul(
                                dst,
                                agT[qt][qo : qo + 64, 0:A],
                                qkT[6 + qt][qo : qo + 64, n0 : n0 + nsz],
                                start=True,
                                stop=True,
                            )
                    e2 = wpool.tile([128, N], BF, tag="e2")
                    nc.scalar.activation(
                        e2[0:113, 0:784].rearrange("p (c x) -> p c x", c=2),
                        scA[0:113].rearrange("p (c x) -> p c x", c=2)[:, :, 0:392],
                        AF.Exp,
                        scale=SCALE23,
                    )
                    nc.scalar.activation(
                        e2[0:113, 784:1176], sc2[0:113, 0:392], AF.Exp, scale=SCALE23
                    )
                    av = wpool.tile([128, 65], BF, tag="avx")
                    nc.vector.memset(av[:, 64:65], 1.0)
                    # transposes: both heads at once ([113, ktsz] -> [ktsz, 113])
                    e2T = []
                    for half in range(2):
                        trp = ppool.tile([128, 5 * TSP], BF, tag="P", name="psTr", bufs=4)
                        for kk in range(5):
                            kt = 5 * half + kk
                            p0, psz = POS_T[kt]
                            nc.tensor.transpose(
                                trp[0:psz, TSP * kk : TSP * kk + 113],
                                e2[0:113, p0 : p0 + psz],
                                ident[0:113, 0:113],
                            )
                        eT = wpool.tile([128, 5 * TSP], BF, tag="e2T")
                        e2T.append(eT)
                        nc.vector.tensor_copy(eT[:, 0 : 5 * TSP], trp[:, 0 : 5 * TSP])
                    pv2 = ppool.tile([128, 512], F32, tag="P", name="psP", bufs=4)
                    for hp in range(2):
                        h = 2 * p2 + hp
                        o = 64 * hp
                        for kt, (p0, psz) in enumerate(POS_T):
                            eT = e2T[kt // 5]
                            cof = TSP * (kt % 5) + 64 * hp
                            nc.tensor.matmul(
                                pv2[o : o + 49, 0:65],
                                eT[0:psz, cof : cof + 49],
                                v_ext[kt][0:psz, 65 * h : 65 * h + 65],
                                start=(kt == 0),
                                stop=(kt == 9),
                            )
                    avr = wpool.tile([128, 1], F32, tag="avr")
                    nc.vector.reciprocal(avr[0:113, 0:1], pv2[0:113, 64:65])
                    nc.vector.tensor_scalar_mul(
                        av[0:113, 0:64], pv2[0:113, 0:64], avr[0:113, 0:1]
                    )

                    # ---- stage 3 for this pair ----
                    scB = ppool.tile([128, 2 * 512], F32, tag="A", name="psA")
                    for hp in range(2):
                        h = 2 * p2 + hp
                        qt, qo = h // 2, (h % 2) * 64
                        for cc in range(2):
                            nc.tensor.matmul(
                                scB[64 * hp : 64 * hp + 49, 512 * cc : 512 * cc + 392],
                                agT[qt][qo : qo + 64, 0:A],
                                qkT[qt][
                                    qo : qo + 64, N_MT + 392 * cc : N_MT + 392 * (cc + 1)
                                ],
                                start=True,
                                stop=True,
                            )
                    e3 = wpool.tile([128, N_S], BF, tag="e3")
                    nc.scalar.activation(
                        e3[0:113, 0:N_S].rearrange("p (c x) -> p c x", c=2),
                        scB[0:113].rearrange("p (c x) -> p c x", c=2)[:, :, 0:392],
                        AF.Exp,
                        scale=SCALE23,
                    )
                    for hp in range(2):
                        h = 2 * p2 + hp
                        qt, qo = h // 2, (h % 2) * 64
                        for cc in range(2):
                            pv = ppool.tile([128, 512], F32, tag="P", name="psP", bufs=4)
                            nc.tensor.matmul(
                                pv[0:65, 0:392],
                                av[64 * hp : 64 * hp + 49, 0:65],
                                e3[64 * hp : 64 * hp + 49, 392 * cc : 392 * (cc + 1)],
                                start=True,
                                stop=True,
                            )
                            se = wpool.tile([1, 392], F32, tag="se")
                            nc.vector.tensor_copy(se[:, 0:392], pv[64:65, 0:392])
                            rc = wpool.tile([1, 392], F32, tag="rc")
                            nc.vector.reciprocal_approx_fast(
                                out=rc[:, 0:392], in_=se[:, 0:392]
                            )
                            bc = wpool.tile([64, 392], F32, tag="bc")
                            nc.gpsimd.partition_broadcast(bc[:], rc[0:1, 0:392])
                            nc.vector.tensor_mul(
                                aoT[qt][
                                    qo : qo + 64, N_MT + 392 * cc : N_MT + 392 * (cc + 1)
                                ],
                                pv[0:64, 0:392],
                                bc[:],
                            )

                # ---- proj: out[pos, c] = aoT.T @ wpjT + bias ----
                for pt, (p0, psz) in enumerate(POS_T):
                    ps = ppool.tile([128, 2 * 512], F32, tag="A", name="psA")
                    for c0, csz in [(0, 512), (512, 256)]:
                        for kt in range(6):
                            nc.tensor.matmul(
                                ps[0:psz, c0 : c0 + csz],
                                aoT[kt][:, p0 : p0 + psz],
                                wp[kt][:, c0 : c0 + csz],
                                start=(kt == 0),
                                stop=False,
                            )
                        nc.tensor.matmul(
                            ps[0:psz, c0 : c0 + csz],
                            ones_r[0:1, 0:psz],
                            bpj_r[0:1, c0 : c0 + csz],
                            start=False,
                            stop=True,
                        )
                    ob = wpool.tile([128, C], F32, tag="osb")
                    nc.vector.tensor_copy(ob[0:psz, :], ps[0:psz, 0:C])
                    nc.sync.dma_start(out_d[b, p0 : p0 + psz, :], ob[0:psz, :])

    nc.compile()
    return nc


_PROGRAM = None


def _get_program():
    global _PROGRAM
    if _PROGRAM is None:
        _PROGRAM = build_program()
    return _PROGRAM


def kernel(x, qkv_w, qkv_b, proj_w, proj_b, t_h=14, t_w=14, s_h=28, s_w=28, **kw):
    x = np.asarray(x, dtype=np.float32)
    bf = ml_dtypes.bfloat16
    xT = np.ascontiguousarray(x.transpose(0, 2, 1)).astype(bf)  # [B, C, N]
    wqkT = np.ascontiguousarray(np.asarray(qkv_w, dtype=np.float32).T).astype(bf)
    wpjT = np.ascontiguousarray(np.asarray(proj_w, dtype=np.float32).T)
    bqk = np.asarray(qkv_b, dtype=np.float32).reshape(1, -1).astype(bf)
    bpj = np.asarray(proj_b, dtype=np.float32).reshape(1, -1)

    nc = _get_program()
    in_maps = []
    for c in range(N_CORES):
        in_maps.append(
            {
                "xT": np.ascontiguousarray(xT[c * NB : (c + 1) * NB]),
                "wqkT": wqkT,
                "wpjT": wpjT,
                "bqk": bqk,
                "bpj": bpj,
            }
        )
    res = bass_utils.run_bass_kernel_spmd(nc, in_maps, core_ids=list(range(N_CORES)))
    out = np.concatenate([res.results[c]["out"] for c in range(N_CORES)], axis=0)
    return out.astype(np.float32)


if __name__ == "__main__":
    build_program()
    print("program built OK")


# revision 32
# speedup vs baseline: 1.0003x; 1.0003x over previous
"""AgentAttention Trainium2 kernel: 8-core data-parallel over batch.

Layouts (per core, 4 batches):
  xT      [4, 768, 1176] bf16  (c-major x)
  qkT     c-major q,k: 12 sbuf tiles [128, 1176] (tiles 0-5 = q rows, 6-11 = k rows)
  v_ext   pos-major v with per-head ones column (col 64): 10 tiles [128, 12*65]
  agT     pooled agent tokens (sums over 4x4 blocks), c-major [128, 49] x6
  aoT     c-major attention output (fp32r) [128, 1176] x6 -> proj -> pos-major out
Matmuls bf16 (fp32r for proj), fp32 psum. Softmax scale folded into ACT exp
scale (0.125 stage1; 0.125/16 stages 2/3 -- agent tokens are pooled SUMS).
Biases are added via K=1 matmul accumulation into psum.
PSUM: tag A = [128,1024] (2 banks) x2, tag P = [128,512] (1 bank) x4.
"""

import sys

sys.path.insert(0, "/opt/trn_rl_repo")

import numpy as np
import ml_dtypes

import concourse.bass as bass
import concourse.mybir as mybir
import concourse.tile as tile
from concourse import bacc, bass_utils
from concourse.masks import make_identity

BF = mybir.dt.bfloat16
F32 = mybir.dt.float32
FR = mybir.dt.float32r
AF = mybir.ActivationFunctionType
AX = mybir.AxisListType

N_CORES = 8
B, N, C = 32, 1176, 768
NB = B // N_CORES
H, HD = 12, 64
N_MT, N_S = 392, 784
A = 49
SCALE1 = 0.125
SCALE23 = 0.125 / 16.0

POS_T = [(pt * 128, min(128, N - pt * 128)) for pt in range(10)]
KEY1_T = [(0, 128), (128, 128), (256, 128), (384, 8)]
NCHUNK = [(0, 392), (392, 392), (784, 392)]
TSP = 116  # transpose chunk col spacing (>=113, even)


def build_program():
    nc = bacc.Bacc("TRN2", debug=False, num_devices=N_CORES)

    xT_d = nc.dram_tensor("xT", [NB, C, N], BF, kind="ExternalInput").ap()
    wqkT_d = nc.dram_tensor("wqkT", [C, 3 * C], BF, kind="ExternalInput").ap()
    wpjT_d = nc.dram_tensor("wpjT", [C, C], F32, kind="ExternalInput").ap()
    bqk_d = nc.dram_tensor("bqk", [1, 3 * C], BF, kind="ExternalInput").ap()
    bpj_d = nc.dram_tensor("bpj", [1, C], F32, kind="ExternalInput").ap()
    out_d = nc.dram_tensor("out", [NB, N, C], F32, kind="ExternalOutput").ap()

    with tile.TileContext(nc) as tc, nc.allow_low_precision(reason="attn bf16"):
        with (
            tc.tile_pool(name="const", bufs=1) as cpool,
            tc.tile_pool(name="work", bufs=2) as wpool,
            tc.tile_pool(name="hold", bufs=1) as hpool,
            tc.tile_pool(name="psum", bufs=2, space="PSUM") as ppool,
        ):
            # ---- one-time constants/weights ----
            wq = [
                cpool.tile([128, 3 * C], BF, tag=f"wq{i}", name=f"wq{i}")
                for i in range(6)
            ]
            wp = [
                cpool.tile([128, C], FR, tag=f"wp{i}", name=f"wp{i}") for i in range(6)
            ]
            for i in range(6):
                nc.sync.dma_start(wq[i][:], wqkT_d[128 * i : 128 * (i + 1), :])
                wps = wpool.tile([128, C], F32, tag="wps", name="wps")
                nc.sync.dma_start(wps[:], wpjT_d[128 * i : 128 * (i + 1), :])
                nc.vector.tensor_copy(wp[i][:], wps[:])
            sb_bqk = cpool.tile([1, 3 * C], BF, tag="bqk")
            nc.sync.dma_start(sb_bqk[:], bqk_d[:])
            ones = cpool.tile([1, 512], BF, tag="ones")
            nc.gpsimd.memset(ones[:], 1.0)
            ones_r = cpool.tile([1, 512], FR, tag="ones_r")
            onesf = wpool.tile([1, 512], F32, tag="onesf", name="onesf")
            nc.gpsimd.memset(onesf[:], 1.0)
            nc.vector.tensor_copy(ones_r[:], onesf[:])
            bpj_r = cpool.tile([1, C], FR, tag="bpj_r")
            bpjf = wpool.tile([1, C], F32, tag="bpjf", name="bpjf")
            nc.sync.dma_start(bpjf[:], bpj_d[:])
            nc.vector.tensor_copy(bpj_r[:], bpjf[:])
            ident = cpool.tile([128, 128], BF, tag="ident")
            make_identity(nc, ident[:])

            for b in range(NB):
                # ---- load xT ----
                xT = [
                    wpool.tile([128, N], BF, tag=f"xT{i}", name=f"xT{i}", bufs=1)
                    for i in range(6)
                ]
                for i in range(6):
                    nc.sync.dma_start(xT[i][:], xT_d[b, 128 * i : 128 * (i + 1), :])

                # ---- phase Q: qkT c-major (q,k rows), bias via K=1 matmul ----
                qkT = [None] * 12
                for m in [0, 6, 1, 7, 2, 8, 3, 9, 4, 10, 5, 11]:
                    ps = ppool.tile([128, 2 * 512], F32, tag="A", name="psA")
                    ps2 = ppool.tile([128, 512], F32, tag="P", name="psP", bufs=4)
                    for j, (n0, nsz) in enumerate(NCHUNK):
                        dst = ps[:, 512 * j : 512 * j + nsz] if j < 2 else ps2[:, 0:nsz]
                        for kt in range(6):
                            nc.tensor.matmul(
                                dst,
                                wq[kt][:, 128 * m : 128 * (m + 1)],
                                xT[kt][:, n0 : n0 + nsz],
                                start=(kt == 0),
                                stop=False,
                            )
                        nc.tensor.matmul(
                            dst,
                            sb_bqk[0:1, 128 * m : 128 * (m + 1)],
                            ones[0:1, 0:nsz],
                            start=False,
                            stop=True,
                        )
                    t = hpool.tile([128, N], BF, tag=f"qkT{m}", name=f"qkT{m}")
                    qkT[m] = t
                    nc.scalar.copy(
                        t[:, 0:784].rearrange("p (c x) -> p c x", c=2),
                        ps[:].rearrange("p (c x) -> p c x", c=2)[:, :, 0:392],
                    )
                    nc.scalar.copy(t[:, 784:1176], ps2[:, 0:392])

                # ---- phase V: pos-major v_ext with ones cols ----
                v_ext = []
                for pt, (p0, psz) in enumerate(POS_T):
                    ps = ppool.tile([128, 2 * 512], F32, tag="A", name="psA")
                    for c0, csz in [(0, 512), (512, 256)]:
                        for kt in range(6):
                            nc.tensor.matmul(
                                ps[0:psz, c0 : c0 + csz],
                                xT[kt][:, p0 : p0 + psz],
                                wq[kt][:, 2 * C + c0 : 2 * C + c0 + csz],
                                start=(kt == 0),
                                stop=False,
                            )
                        nc.tensor.matmul(
                            ps[0:psz, c0 : c0 + csz],
                            ones[0:1, 0:psz],
                            sb_bqk[0:1, 2 * C + c0 : 2 * C + c0 + csz],
                            start=False,
                            stop=True,
                        )
                    vt = hpool.tile([128, H * 65], BF, tag=f"vx{pt}", name=f"vx{pt}")
                    v_ext.append(vt)
                    if b == 0:
                        # bufs=1 slot memory persists across batches; evac only
                        # writes the 64 v columns, so ones survive
                        nc.vector.memset(
                            vt[:].rearrange("p (h e) -> p h e", e=65)[:, :, 64:65], 1.0
                        )
                    nc.scalar.copy(
                        vt[0:psz].rearrange("p (h e) -> p h e", e=65)[:, :, 0:64],
                        ps[0:psz, 0:768].rearrange("p (h d) -> p h d", d=64),
                    )

                # ---- agent pooling: sum 4x4 blocks of q_s -> agT (c-major) ----
                agT = []
                for ct in range(6):
                    t1 = wpool.tile([128, 196], F32, tag="t1")
                    qs = qkT[ct][:, N_MT:N]  # [128, 784], idx = i*28 + aj*4 + dj
                    q4 = qs.rearrange("p (x dj) -> p x dj", dj=4)
                    nc.gpsimd.tensor_add(t1[:, 0:196], q4[:, :, 0:1], q4[:, :, 1:2])
                    nc.gpsimd.tensor_add(t1[:, 0:196], t1[:, 0:196], q4[:, :, 2:3])
                    nc.gpsimd.tensor_add(t1[:, 0:196], t1[:, 0:196], q4[:, :, 3:4])
                    ag = wpool.tile([128, A], BF, tag=f"ag{ct}", name=f"ag{ct}")
                    agT.append(ag)
                    # t1 idx = 28*ai + 7*di + aj -> view (ai, aj, di)
                    t4 = t1[:, 0:196].rearrange("p (ai di aj) -> p ai aj di", ai=7, di=4)
                    t2 = wpool.tile([128, A], F32, tag="t2")
                    nc.gpsimd.tensor_add(t2[:, 0:A], t4[:, :, :, 0:1], t4[:, :, :, 1:2])
                    nc.gpsimd.tensor_add(t2[:, 0:A], t2[:, 0:A], t4[:, :, :, 2:3])
                    nc.gpsimd.tensor_add(ag[:, 0:A], t2[:, 0:A], t4[:, :, :, 3:4])

                # attention output accumulates here (c-major, fp32r)
                aoT = [
                    hpool.tile([128, N], FR, tag=f"ao{i}", name=f"ao{i}")
                    for i in range(6)
                ]

                # ---- stage 1: one head ----
                def stage1(h):
                    qt, qo = h // 2, (h % 2) * 64
                    scA = ppool.tile([128, 2 * 512], F32, tag="A", name="psA")
                    sc2 = ppool.tile([128, 512], F32, tag="P", name="psP", bufs=4)
                    scP = ppool.tile([128, 512], F32, tag="P", name="psP", bufs=4)
                    for j, (k0, ksz) in enumerate(KEY1_T):
                        dst = (
                            scA[0:ksz, 512 * j : 512 * j + 392]
                            if j < 2
                            else (sc2[0:ksz, 0:392] if j == 2 else scP[0:ksz, 0:392])
                        )
                        nc.tensor.matmul(
                            dst,
                            qkT[6 + qt][qo : qo + 64, k0 : k0 + ksz],
                            qkT[qt][qo : qo + 64, 0:N_MT],
                            start=True,
                            stop=True,
                        )
                    e1 = wpool.tile([128, 4 * 392], BF, tag="e1")
                    nc.scalar.activation(
                        e1[:, 0:784].rearrange("p (c x) -> p c x", c=2),
                        scA[:].rearrange("p (c x) -> p c x", c=2)[:, :, 0:392],
                        AF.Exp,
                        scale=SCALE1,
                    )
                    nc.scalar.activation(
                        e1[:, 784:1176], sc2[:, 0:392], AF.Exp, scale=SCALE1
                    )
                    nc.scalar.activation(
                        e1[0:8, 1176:1568], scP[0:8, 0:392], AF.Exp, scale=SCALE1
                    )
                    pv = ppool.tile([128, 512], F32, tag="P", name="psP", bufs=4)
                    for j, (k0, ksz) in enumerate(KEY1_T):
                        nc.tensor.matmul(
                            pv[0:65, 0:392],
                            v_ext[j][0:ksz, 65 * h : 65 * h + 65],
                            e1[0:ksz, 392 * j : 392 * j + 392],
                            start=(j == 0),
                            stop=(j == 3),
                        )
                    se = wpool.tile([1, 392], F32, tag="se")
                    nc.vector.tensor_copy(se[:, 0:392], pv[64:65, 0:392])
                    rc = wpool.tile([1, 392], F32, tag="rc")
                    nc.vector.reciprocal_approx_fast(out=rc[:, 0:392], in_=se[:, 0:392])
                    bc = wpool.tile([64, 392], F32, tag="bc")
                    nc.gpsimd.partition_broadcast(bc[:], rc[0:1, 0:392])
                    nc.vector.tensor_mul(
                        aoT[qt][qo : qo + 64, 0:N_MT], pv[0:64, 0:392], bc[:]
                    )

                # ---- stages 1+2+3 interleaved per head pair ----
                for p2 in range(6):
                    stage1(2 * p2)
                    stage1(2 * p2 + 1)
                    # stage 2 scores (both heads packed at partition 0/64)
                    scA = ppool.tile([128, 2 * 512], F32, tag="A", name="psA")
                    sc2 = ppool.tile([128, 512], F32, tag="P", name="psP", bufs=4)
                    for hp in range(2):
                        h = 2 * p2 + hp
                        qt, qo = h // 2, (h % 2) * 64
                        for j, (n0, nsz) in enumerate(NCHUNK):
                            dst = (
                                scA[64 * hp : 64 * hp + 49, 512 * j : 512 * j + nsz]
                                if j < 2
                                else sc2[64 * hp : 64 * hp + 49, 0:nsz]
                            )
                            nc.tensor.matmul(
                                dst,
                                agT[qt][qo : qo + 64, 0:A],
                                qkT[6 + qt][qo : qo + 64, n0 : n0 + nsz],
                                start=True,
                                stop=True,
                            )
                    e2 = wpool.tile([128, N], BF, tag="e2")
                    nc.scalar.activation(
                        e2[0:113, 0:784].rearrange("p (c x) -> p c x", c=2),
                        scA[0:113].rearrange("p (c x) -> p c x", c=2)[:, :, 0:392],
                        AF.Exp,
                        scale=SCALE23,
                    )
                    nc.scalar.activation(
                        e2[0:113, 784:1176], sc2[0:113, 0:392], AF.Exp, scale=SCALE23
                    )
                    av = wpool.tile([128, 65], BF, tag="avx")
                    nc.vector.memset(av[:, 64:65], 1.0)
                    # transposes: both heads at once ([113, ktsz] -> [ktsz, 113])
                    e2T = []
                    for half in range(2):
                        trp = ppool.tile([128, 5 * TSP], BF, tag="P", name="psTr", bufs=4)
                        for kk in range(5):
                            kt = 5 * half + kk
                            p0, psz = POS_T[kt]
                            nc.tensor.transpose(
                                trp[0:psz, TSP * kk : TSP * kk + 113],
                                e2[0:113, p0 : p0 + psz],
                                ident[0:113, 0:113],
                            )
                        eT = wpool.tile([128, 5 * TSP], BF, tag="e2T")
                        e2T.append(eT)
                        nc.vector.tensor_copy(eT[:, 0 : 5 * TSP], trp[:, 0 : 5 * TSP])
                    pv2 = ppool.tile([128, 512], F32, tag="P", name="psP", bufs=4)
                    for hp in range(2):
                        h = 2 * p2 + hp
                        o = 64 * hp
                        for kt, (p0, psz) in enumerate(POS_T):
                            eT = e2T[kt // 5]
                            cof = TSP * (kt % 5) + 64 * hp
                            nc.tensor.matmul(
                                pv2[o : o + 49, 0:65],
                                eT[0:psz, cof : cof + 49],
                                v_ext[kt][0:psz, 65 * h : 65 * h + 65],
                                start=(kt == 0),
                                stop=(kt == 9),
                            )
                    avr = wpool.tile([128, 1], F32, tag="avr")
                    nc.vector.reciprocal(avr[0:113, 0:1], pv2[0:113, 64:65])
                    nc.vector.tensor_scalar_mul(
                        av[0:113, 0:64], pv2[0:113, 0:64], avr[0:113, 0:1]
                    )

                    # ---- stage 3 for this pair ----
                    scB = ppool.tile([128, 2 * 512], F32, tag="A", name="psA")
                    for hp in range(2):
                        h = 2 * p2 + hp
                        qt, qo = h // 2, (h % 2) * 64
                        for cc in range(2):
                            nc.tensor.matmul(
                                scB[64 * hp : 64 * hp + 49, 512 * cc : 512 * cc + 392],
                                agT[qt][qo : qo + 64, 0:A],
                                qkT[qt][
                                    qo : qo + 64, N_MT + 392 * cc : N_MT + 392 * (cc + 1)
                                ],
                                start=True,
                                stop=True,
                            )
                    e3 = wpool.tile([128, N_S], BF, tag="e3")
                    nc.scalar.activation(
                        e3[0:113, 0:N_S].rearrange("p (c x) -> p c x", c=2),
                        scB[0:113].rearrange("p (c x) -> p c x", c=2)[:, :, 0:392],
                        AF.Exp,
                        scale=SCALE23,
                    )
                    for hp in range(2):
                        h = 2 * p2 + hp
                        qt, qo = h // 2, (h % 2) * 64
                        for cc in range(2):
                            pv = ppool.tile([128, 512], F32, tag="P", name="psP", bufs=4)
                            nc.tensor.matmul(
                                pv[0:65, 0:392],
                                av[64 * hp : 64 * hp + 49, 0:65],
                                e3[64 * hp : 64 * hp + 49, 392 * cc : 392 * (cc + 1)],
                                start=True,
                                stop=True,
                            )
                            se = wpool.tile([1, 392], F32, tag="se")
                            nc.vector.tensor_copy(se[:, 0:392], pv[64:65, 0:392])
                            rc = wpool.tile([1, 392], F32, tag="rc")
                            nc.vector.reciprocal_approx_fast(
                                out=rc[:, 0:392], in_=se[:, 0:392]
                            )
                            bc = wpool.tile([64, 392], F32, tag="bc")
                            nc.gpsimd.partition_broadcast(bc[:], rc[0:1, 0:392])
                            nc.vector.tensor_mul(
                                aoT[qt][
                                    qo : qo + 64, N_MT + 392 * cc : N_MT + 392 * (cc + 1)
                                ],
                                pv[0:64, 0:392],
                                bc[:],
                            )

                # ---- proj: out[pos, c] = aoT.T @ wpjT + bias ----
                for pt, (p0, psz) in enumerate(POS_T):
                    ps = ppool.tile([128, 2 * 512], F32, tag="A", name="psA")
                    for c0, csz in [(0, 512), (512, 256)]:
                        for kt in range(6):
                            nc.tensor.matmul(
                                ps[0:psz, c0 : c0 + csz],
                                aoT[kt][:, p0 : p0 + psz],
                                wp[kt][:, c0 : c0 + csz],
                                start=(kt == 0),
                                stop=False,
                            )
                        nc.tensor.matmul(
                            ps[0:psz, c0 : c0 + csz],
                            ones_r[0:1, 0:psz],
                            bpj_r[0:1, c0 : c0 + csz],
                            start=False,
                            stop=True,
                        )
                    ob = wpool.tile([128, C], F32, tag="osb")
                    nc.vector.tensor_copy(ob[0:psz, :], ps[0:psz, 0:C])
                    nc.sync.dma_start(out_d[b, p0 : p0 + psz, :], ob[0:psz, :])

    nc.compile()
    return nc


_PROGRAM = None


def _get_program():
    global _PROGRAM
    if _PROGRAM is None:
        _PROGRAM = build_program()
    return _PROGRAM


def kernel(x, qkv_w, qkv_b, proj_w, proj_b, t_h=14, t_w=14, s_h=28, s_w=28, **kw):
    x = np.asarray(x, dtype=np.float32)
    bf = ml_dtypes.bfloat16
    xT = np.ascontiguousarray(x.transpose(0, 2, 1)).astype(bf)  # [B, C, N]
    wqkT = np.ascontiguousarray(np.asarray(qkv_w, dtype=np.float32).T).astype(bf)
    wpjT = np.ascontiguousarray(np.asarray(proj_w, dtype=np.float32).T)
    bqk = np.asarray(qkv_b, dtype=np.float32).reshape(1, -1).astype(bf)
    bpj = np.asarray(proj_b, dtype=np.float32).reshape(1, -1)

    nc = _get_program()
    in_maps = []
    for c in range(N_CORES):
        in_maps.append(
            {
                "xT": np.ascontiguousarray(xT[c * NB : (c + 1) * NB]),
                "wqkT": wqkT,
                "wpjT": wpjT,
                "bqk": bqk,
                "bpj": bpj,
            }
        )
    res = bass_utils.run_bass_kernel_spmd(nc, in_maps, core_ids=list(range(N_CORES)))
    out = np.concatenate([res.results[c]["out"] for c in range(N_CORES)], axis=0)
    return out.astype(np.float32)


if __name__ == "__main__":
    build_program()
    print("program built OK")


# revision 33
# speedup vs baseline: 1.2255x; 1.2251x over previous
"""AgentAttention Trainium2 kernel: 8-core data-parallel over batch.

Layouts (per core, 4 batches):
  xT      [4, 768, 1176] bf16  (c-major x)
  qkT     c-major q,k: 12 sbuf tiles [128, 1176] (tiles 0-5 = q rows, 6-11 = k rows)
  v_ext   pos-major v with per-head ones column (col 64): 10 tiles [128, 12*65]
  agT     pooled agent tokens (sums over 4x4 blocks), c-major [128, 49] x6
  aoT     c-major attention output (fp32r) [128, 1176] x6 -> proj -> pos-major out
Matmuls bf16 (fp32r for proj), fp32 psum. Softmax scale folded into ACT exp
scale (0.125 stage1; 0.125/16 stages 2/3 -- agent tokens are pooled SUMS).
Biases are added via K=1 matmul accumulation into psum.
PSUM: tag A = [128,1024] (2 banks) x2, tag P = [128,512] (1 bank) x4.
"""

import sys

sys.path.insert(0, "/opt/trn_rl_repo")

import numpy as np
import ml_dtypes

import concourse.bass as bass
import concourse.mybir as mybir
import concourse.tile as tile
from concourse import bacc, bass_utils
from concourse.masks import make_identity

BF = mybir.dt.bfloat16
F32 = mybir.dt.float32
FR = mybir.dt.float32r
AF = mybir.ActivationFunctionType
AX = mybir.AxisListType

N_CORES = 8
B, N, C = 32, 1176, 768
NB = B // N_CORES
H, HD = 12, 64
N_MT, N_S = 392, 784
A = 49
SCALE1 = 0.125
SCALE23 = 0.125 / 16.0

POS_T = [(pt * 128, min(128, N - pt * 128)) for pt in range(10)]
KEY1_T = [(0, 128), (128, 128), (256, 128), (384, 8)]
NCHUNK = [(0, 392), (392, 392), (784, 392)]
TSP = 116  # transpose chunk col spacing (>=113, even)


def build_program():
    nc = bacc.Bacc("TRN2", debug=False, num_devices=N_CORES)

    xT_d = nc.dram_tensor("xT", [NB, C, N], BF, kind="ExternalInput").ap()
    wqkT_d = nc.dram_tensor("wqkT", [C, 3 * C], BF, kind="ExternalInput").ap()
    wpjT_d = nc.dram_tensor("wpjT", [C, C], F32, kind="ExternalInput").ap()
    bqk_d = nc.dram_tensor("bqk", [1, 3 * C], BF, kind="ExternalInput").ap()
    bpj_d = nc.dram_tensor("bpj", [1, C], F32, kind="ExternalInput").ap()
    out_d = nc.dram_tensor("out", [NB, N, C], F32, kind="ExternalOutput").ap()

    with tile.TileContext(nc) as tc, nc.allow_low_precision(reason="attn bf16"):
        with (
            tc.tile_pool(name="const", bufs=1) as cpool,
            tc.tile_pool(name="work", bufs=2) as wpool,
            tc.tile_pool(name="hold", bufs=1) as hpool,
            tc.tile_pool(name="psum", bufs=2, space="PSUM") as ppool,
        ):
            # ---- one-time constants/weights ----
            wq = [
                cpool.tile([128, 3 * C], BF, tag=f"wq{i}", name=f"wq{i}")
                for i in range(6)
            ]
            wp = [
                cpool.tile([128, C], FR, tag=f"wp{i}", name=f"wp{i}") for i in range(6)
            ]
            for i in range(6):
                nc.sync.dma_start(wq[i][:], wqkT_d[128 * i : 128 * (i + 1), :])
                wps = wpool.tile([128, C], F32, tag="wps", name="wps")
                nc.sync.dma_start(wps[:], wpjT_d[128 * i : 128 * (i + 1), :])
                nc.vector.tensor_copy(wp[i][:], wps[:])
            sb_bqk = cpool.tile([1, 3 * C], BF, tag="bqk")
            nc.sync.dma_start(sb_bqk[:], bqk_d[:])
            ones = cpool.tile([1, 512], BF, tag="ones")
            nc.gpsimd.memset(ones[:], 1.0)
            ones_r = cpool.tile([1, 512], FR, tag="ones_r")
            onesf = wpool.tile([1, 512], F32, tag="onesf", name="onesf")
            nc.gpsimd.memset(onesf[:], 1.0)
            nc.vector.tensor_copy(ones_r[:], onesf[:])
            bpj_r = cpool.tile([1, C], FR, tag="bpj_r")
            bpjf = wpool.tile([1, C], F32, tag="bpjf", name="bpjf")
            nc.sync.dma_start(bpjf[:], bpj_d[:])
            nc.vector.tensor_copy(bpj_r[:], bpjf[:])
            ident = cpool.tile([128, 128], BF, tag="ident")
            make_identity(nc, ident[:])

            for b in range(NB):
                # ---- load xT ----
                xT = [
                    wpool.tile([128, N], BF, tag=f"xT{i}", name=f"xT{i}", bufs=1)
                    for i in range(6)
                ]
                for i in range(6):
                    nc.sync.dma_start(xT[i][:], xT_d[b, 128 * i : 128 * (i + 1), :])

                # ---- phase Q: qkT c-major (q,k rows), bias via K=1 matmul ----
                qkT = [None] * 12
                for m in [0, 6, 1, 7, 2, 8, 3, 9, 4, 10, 5, 11]:
                    ps = ppool.tile([128, 2 * 512], F32, tag="A", name="psA")
                    ps2 = ppool.tile([128, 512], F32, tag="P", name="psP", bufs=4)
                    for j, (n0, nsz) in enumerate(NCHUNK):
                        dst = ps[:, 512 * j : 512 * j + nsz] if j < 2 else ps2[:, 0:nsz]
                        for kt in range(6):
                            nc.tensor.matmul(
                                dst,
                                wq[kt][:, 128 * m : 128 * (m + 1)],
                                xT[kt][:, n0 : n0 + nsz],
                                start=(kt == 0),
                                stop=False,
                            )
                        nc.tensor.matmul(
                            dst,
                            sb_bqk[0:1, 128 * m : 128 * (m + 1)],
                            ones[0:1, 0:nsz],
                            start=False,
                            stop=True,
                        )
                    t = hpool.tile([128, N], BF, tag=f"qkT{m}", name=f"qkT{m}")
                    qkT[m] = t
                    nc.scalar.copy(
                        t[:, 0:784].rearrange("p (c x) -> p c x", c=2),
                        ps[:].rearrange("p (c x) -> p c x", c=2)[:, :, 0:392],
                    )
                    nc.scalar.copy(t[:, 784:1176], ps2[:, 0:392])

                # ---- phase V: pos-major v_ext with ones cols ----
                v_ext = []
                for pt, (p0, psz) in enumerate(POS_T):
                    ps = ppool.tile([128, 2 * 512], F32, tag="A", name="psA")
                    for c0, csz in [(0, 512), (512, 256)]:
                        for kt in range(6):
                            nc.tensor.matmul(
                                ps[0:psz, c0 : c0 + csz],
                                xT[kt][:, p0 : p0 + psz],
                                wq[kt][:, 2 * C + c0 : 2 * C + c0 + csz],
                                start=(kt == 0),
                                stop=False,
                            )
                        nc.tensor.matmul(
                            ps[0:psz, c0 : c0 + csz],
                            ones[0:1, 0:psz],
                            sb_bqk[0:1, 2 * C + c0 : 2 * C + c0 + csz],
                            start=False,
                            stop=True,
                        )
                    vt = hpool.tile([128, H * 65], BF, tag=f"vx{pt}", name=f"vx{pt}")
                    v_ext.append(vt)
                    if b == 0:
                        # bufs=1 slot memory persists across batches; evac only
                        # writes the 64 v columns, so ones survive
                        nc.vector.memset(
                            vt[:].rearrange("p (h e) -> p h e", e=65)[:, :, 64:65], 1.0
                        )
                    nc.scalar.copy(
                        vt[0:psz].rearrange("p (h e) -> p h e", e=65)[:, :, 0:64],
                        ps[0:psz, 0:768].rearrange("p (h d) -> p h d", d=64),
                    )

                # ---- agent pooling: sum 4x4 blocks of q_s -> agT (c-major) ----
                agT = []
                for ct in range(6):
                    t1 = wpool.tile([128, 196], F32, tag="t1")
                    qs = qkT[ct][:, N_MT:N]  # [128, 784], idx = i*28 + aj*4 + dj
                    q4 = qs.rearrange("p (x dj) -> p x dj", dj=4)
                    nc.gpsimd.tensor_add(t1[:, 0:196], q4[:, :, 0:1], q4[:, :, 1:2])
                    nc.gpsimd.tensor_add(t1[:, 0:196], t1[:, 0:196], q4[:, :, 2:3])
                    nc.gpsimd.tensor_add(t1[:, 0:196], t1[:, 0:196], q4[:, :, 3:4])
                    ag = wpool.tile([128, A], BF, tag=f"ag{ct}", name=f"ag{ct}")
                    agT.append(ag)
                    # t1 idx = 28*ai + 7*di + aj -> view (ai, aj, di)
                    t4 = t1[:, 0:196].rearrange("p (ai di aj) -> p ai aj di", ai=7, di=4)
                    t2 = wpool.tile([128, A], F32, tag="t2")
                    nc.gpsimd.tensor_add(t2[:, 0:A], t4[:, :, :, 0:1], t4[:, :, :, 1:2])
                    nc.gpsimd.tensor_add(t2[:, 0:A], t2[:, 0:A], t4[:, :, :, 2:3])
                    nc.gpsimd.tensor_add(ag[:, 0:A], t2[:, 0:A], t4[:, :, :, 3:4])

                # attention output accumulates here (c-major, fp32r)
                aoT = [
                    hpool.tile([128, N], FR, tag=f"ao{i}", name=f"ao{i}")
                    for i in range(6)
                ]

                # ---- stage 1: one head ----
                def stage1(h):
                    qt, qo = h // 2, (h % 2) * 64
                    scA = ppool.tile([128, 2 * 512], F32, tag="A", name="psA")
                    sc2 = ppool.tile([128, 512], F32, tag="P", name="psP", bufs=4)
                    scP = ppool.tile([128, 512], F32, tag="P", name="psP", bufs=4)
                    for j, (k0, ksz) in enumerate(KEY1_T):
                        dst = (
                            scA[0:ksz, 512 * j : 512 * j + 392]
                            if j < 2
                            else (sc2[0:ksz, 0:392] if j == 2 else scP[0:ksz, 0:392])
                        )
                        nc.tensor.matmul(
                            dst,
                            qkT[6 + qt][qo : qo + 64, k0 : k0 + ksz],
                            qkT[qt][qo : qo + 64, 0:N_MT],
                            start=True,
                            stop=True,
                        )
                    e1 = [
                        wpool.tile([128, 392], BF, tag="e1", name="e1", bufs=8)
                        for _ in range(4)
                    ]
                    nc.scalar.activation(
                        e1[0][:, 0:392], scA[:, 0:392], AF.Exp, scale=SCALE1
                    )
                    nc.scalar.activation(
                        e1[1][:, 0:392], scA[:, 512:904], AF.Exp, scale=SCALE1
                    )
                    nc.scalar.activation(
                        e1[2][:, 0:392], sc2[:, 0:392], AF.Exp, scale=SCALE1
                    )
                    nc.scalar.activation(
                        e1[3][0:8, 0:392], scP[0:8, 0:392], AF.Exp, scale=SCALE1
                    )
                    pv = ppool.tile([128, 512], F32, tag="P", name="psP", bufs=4)
                    for j, (k0, ksz) in enumerate(KEY1_T):
                        nc.tensor.matmul(
                            pv[0:65, 0:392],
                            v_ext[j][0:ksz, 65 * h : 65 * h + 65],
                            e1[j][0:ksz, 0:392],
                            start=(j == 0),
                            stop=(j == 3),
                        )
                    se = wpool.tile([1, 392], F32, tag="se")
                    nc.vector.tensor_copy(se[:, 0:392], pv[64:65, 0:392])
                    rc = wpool.tile([1, 392], F32, tag="rc")
                    nc.vector.reciprocal_approx_fast(out=rc[:, 0:392], in_=se[:, 0:392])
                    bc = wpool.tile([64, 392], F32, tag="bc")
                    nc.gpsimd.partition_broadcast(bc[:], rc[0:1, 0:392])
                    nc.vector.tensor_mul(
                        aoT[qt][qo : qo + 64, 0:N_MT], pv[0:64, 0:392], bc[:]
                    )

                # ---- stages 1+2+3 interleaved per head pair ----
                for p2 in range(6):
                    stage1(2 * p2)
                    stage1(2 * p2 + 1)
                    # stage 2 scores (both heads packed at partition 0/64)
                    scA = ppool.tile([128, 2 * 512], F32, tag="A", name="psA")
                    sc2 = ppool.tile([128, 512], F32, tag="P", name="psP", bufs=4)
                    for hp in range(2):
                        h = 2 * p2 + hp
                        qt, qo = h // 2, (h % 2) * 64
                        for j, (n0, nsz) in enumerate(NCHUNK):
                            dst = (
                                scA[64 * hp : 64 * hp + 49, 512 * j : 512 * j + nsz]
                                if j < 2
                                else sc2[64 * hp : 64 * hp + 49, 0:nsz]
                            )
                            nc.tensor.matmul(
                                dst,
                                agT[qt][qo : qo + 64, 0:A],
                                qkT[6 + qt][qo : qo + 64, n0 : n0 + nsz],
                                start=True,
                                stop=True,
                            )
                    e2 = wpool.tile([128, N], BF, tag="e2")
                    nc.scalar.activation(
                        e2[0:113, 0:784].rearrange("p (c x) -> p c x", c=2),
                        scA[0:113].rearrange("p (c x) -> p c x", c=2)[:, :, 0:392],
                        AF.Exp,
                        scale=SCALE23,
                    )
                    nc.scalar.activation(
                        e2[0:113, 784:1176], sc2[0:113, 0:392], AF.Exp, scale=SCALE23
                    )
                    av = wpool.tile([128, 65], BF, tag="avx")
                    nc.vector.memset(av[:, 64:65], 1.0)
                    # transposes: both heads at once ([113, ktsz] -> [ktsz, 113])
                    e2T = []
                    for half in range(2):
                        trp = ppool.tile([128, 5 * TSP], BF, tag="P", name="psTr", bufs=4)
                        for kk in range(5):
                            kt = 5 * half + kk
                            p0, psz = POS_T[kt]
                            nc.tensor.transpose(
                                trp[0:psz, TSP * kk : TSP * kk + 113],
                                e2[0:113, p0 : p0 + psz],
                                ident[0:113, 0:113],
                            )
                        eT = wpool.tile([128, 5 * TSP], BF, tag="e2T")
                        e2T.append(eT)
                        nc.vector.tensor_copy(eT[:, 0 : 5 * TSP], trp[:, 0 : 5 * TSP])
                    pv2 = ppool.tile([128, 512], F32, tag="P", name="psP", bufs=4)
                    for hp in range(2):
                        h = 2 * p2 + hp
                        o = 64 * hp
                        for kt, (p0, psz) in enumerate(POS_T):
                            eT = e2T[kt // 5]
                            cof = TSP * (kt % 5) + 64 * hp
                            nc.tensor.matmul(
                                pv2[o : o + 49, 0:65],
                                eT[0:psz, cof : cof + 49],
                                v_ext[kt][0:psz, 65 * h : 65 * h + 65],
                                start=(kt == 0),
                                stop=(kt == 9),
                            )
                    avr = wpool.tile([128, 1], F32, tag="avr")
                    nc.vector.reciprocal(avr[0:113, 0:1], pv2[0:113, 64:65])
                    nc.vector.tensor_scalar_mul(
                        av[0:113, 0:64], pv2[0:113, 0:64], avr[0:113, 0:1]
                    )

                    # ---- stage 3 for this pair ----
                    scB = ppool.tile([128, 2 * 512], F32, tag="A", name="psA")
                    for hp in range(2):
                        h = 2 * p2 + hp
                        qt, qo = h // 2, (h % 2) * 64
                        for cc in range(2):
                            nc.tensor.matmul(
                                scB[64 * hp : 64 * hp + 49, 512 * cc : 512 * cc + 392],
                                agT[qt][qo : qo + 64, 0:A],
                                qkT[qt][
                                    qo : qo + 64, N_MT + 392 * cc : N_MT + 392 * (cc + 1)
                                ],
                                start=True,
                                stop=True,
                            )
                    e3 = [
                        wpool.tile([128, 392], BF, tag="e3", name="e3", bufs=4)
                        for _ in range(2)
                    ]
                    nc.scalar.activation(
                        e3[0][0:113, 0:392], scB[0:113, 0:392], AF.Exp, scale=SCALE23
                    )
                    nc.scalar.activation(
                        e3[1][0:113, 0:392], scB[0:113, 512:904], AF.Exp, scale=SCALE23
                    )
                    for hp in range(2):
                        h = 2 * p2 + hp
                        qt, qo = h // 2, (h % 2) * 64
                        for cc in range(2):
                            pv = ppool.tile([128, 512], F32, tag="P", name="psP", bufs=4)
                            nc.tensor.matmul(
                                pv[0:65, 0:392],
                                av[64 * hp : 64 * hp + 49, 0:65],
                                e3[cc][64 * hp : 64 * hp + 49, 0:392],
                                start=True,
                                stop=True,
                            )
                            se = wpool.tile([1, 392], F32, tag="se")
                            nc.vector.tensor_copy(se[:, 0:392], pv[64:65, 0:392])
                            rc = wpool.tile([1, 392], F32, tag="rc")
                            nc.vector.reciprocal_approx_fast(
                                out=rc[:, 0:392], in_=se[:, 0:392]
                            )
                            bc = wpool.tile([64, 392], F32, tag="bc")
                            nc.gpsimd.partition_broadcast(bc[:], rc[0:1, 0:392])
                            nc.vector.tensor_mul(
                                aoT[qt][
                                    qo : qo + 64, N_MT + 392 * cc : N_MT + 392 * (cc + 1)
                                ],
                                pv[0:64, 0:392],
                                bc[:],
                            )

                # ---- proj: out[pos, c] = aoT.T @ wpjT + bias ----
                for pt, (p0, psz) in enumerate(POS_T):
                    ps = ppool.tile([128, 2 * 512], F32, tag="A", name="psA")
                    for c0, csz in [(0, 512), (512, 256)]:
                        for kt in range(6):
                            nc.tensor.matmul(
                                ps[0:psz, c0 : c0 + csz],
                                aoT[kt][:, p0 : p0 + psz],
                                wp[kt][:, c0 : c0 + csz],
                                start=(kt == 0),
                                stop=False,
                            )
                        nc.tensor.matmul(
                            ps[0:psz, c0 : c0 + csz],
                            ones_r[0:1, 0:psz],
                            bpj_r[0:1, c0 : c0 + csz],
                            start=False,
                            stop=True,
                        )
                    ob = wpool.tile([128, C], F32, tag="osb")
                    nc.vector.tensor_copy(ob[0:psz, :], ps[0:psz, 0:C])
                    nc.sync.dma_start(out_d[b, p0 : p0 + psz, :], ob[0:psz, :])

    nc.compile()
    return nc


_PROGRAM = None


def _get_program():
    global _PROGRAM
    if _PROGRAM is None:
        _PROGRAM = build_program()
    return _PROGRAM


def kernel(x, qkv_w, qkv_b, proj_w, proj_b, t_h=14, t_w=14, s_h=28, s_w=28, **kw):
    x = np.asarray(x, dtype=np.float32)
    bf = ml_dtypes.bfloat16
    xT = np.ascontiguousarray(x.transpose(0, 2, 1)).astype(bf)  # [B, C, N]
    wqkT = np.ascontiguousarray(np.asarray(qkv_w, dtype=np.float32).T).astype(bf)
    wpjT = np.ascontiguousarray(np.asarray(proj_w, dtype=np.float32).T)
    bqk = np.asarray(qkv_b, dtype=np.float32).reshape(1, -1).astype(bf)
    bpj = np.asarray(proj_b, dtype=np.float32).reshape(1, -1)

    nc = _get_program()
    in_maps = []
    for c in range(N_CORES):
        in_maps.append(
            {
                "xT": np.ascontiguousarray(xT[c * NB : (c + 1) * NB]),
                "wqkT": wqkT,
                "wpjT": wpjT,
                "bqk": bqk,
                "bpj": bpj,
            }
        )
    res = bass_utils.run_bass_kernel_spmd(nc, in_maps, core_ids=list(range(N_CORES)))
    out = np.concatenate([res.results[c]["out"] for c in range(N_CORES)], axis=0)
    return out.astype(np.float32)


if __name__ == "__main__":
    build_program()
    print("program built OK")


# revision 34
# speedup vs baseline: 1.2502x; 1.0201x over previous
"""AgentAttention Trainium2 kernel: 8-core data-parallel over batch.

Layouts (per core, 4 batches):
  xT      [4, 768, 1176] bf16  (c-major x)
  qkT     c-major q,k: 12 sbuf tiles [128, 1176] (tiles 0-5 = q rows, 6-11 = k rows)
  v_ext   pos-major v with per-head ones column (col 64): 10 tiles [128, 12*65]
  agT     pooled agent tokens (sums over 4x4 blocks), c-major [128, 49] x6
  aoT     c-major attention output (fp32r) [128, 1176] x6 -> proj -> pos-major out
Matmuls bf16 (fp32r for proj), fp32 psum. Softmax scale folded into ACT exp
scale (0.125 stage1; 0.125/16 stages 2/3 -- agent tokens are pooled SUMS).
Biases are added via K=1 matmul accumulation into psum.
PSUM: tag A = [128,1024] (2 banks) x2, tag P = [128,512] (1 bank) x4.
"""

import sys

sys.path.insert(0, "/opt/trn_rl_repo")

import numpy as np
import ml_dtypes

import concourse.bass as bass
import concourse.mybir as mybir
import concourse.tile as tile
from concourse import bacc, bass_utils
from concourse.masks import make_identity

BF = mybir.dt.bfloat16
F32 = mybir.dt.float32
FR = mybir.dt.float32r
AF = mybir.ActivationFunctionType
AX = mybir.AxisListType

N_CORES = 8
B, N, C = 32, 1176, 768
NB = B // N_CORES
H, HD = 12, 64
N_MT, N_S = 392, 784
A = 49
SCALE1 = 0.125
SCALE23 = 0.125 / 16.0

POS_T = [(pt * 128, min(128, N - pt * 128)) for pt in range(10)]
KEY1_T = [(0, 128), (128, 128), (256, 128), (384, 8)]
NCHUNK = [(0, 392), (392, 392), (784, 392)]
TSP = 116  # transpose chunk col spacing (>=113, even)


def build_program():
    nc = bacc.Bacc("TRN2", debug=False, num_devices=N_CORES)

    xT_d = nc.dram_tensor("xT", [NB, C, N], BF, kind="ExternalInput").ap()
    wqkT_d = nc.dram_tensor("wqkT", [C, 3 * C], BF, kind="ExternalInput").ap()
    wpjT_d = nc.dram_tensor("wpjT", [C, C], F32, kind="ExternalInput").ap()
    bqk_d = nc.dram_tensor("bqk", [1, 3 * C], BF, kind="ExternalInput").ap()
    bpj_d = nc.dram_tensor("bpj", [1, C], F32, kind="ExternalInput").ap()
    out_d = nc.dram_tensor("out", [NB, N, C], F32, kind="ExternalOutput").ap()

    with tile.TileContext(nc) as tc, nc.allow_low_precision(reason="attn bf16"):
        with (
            tc.tile_pool(name="const", bufs=1) as cpool,
            tc.tile_pool(name="work", bufs=2) as wpool,
            tc.tile_pool(name="hold", bufs=1) as hpool,
            tc.tile_pool(name="psum", bufs=2, space="PSUM") as ppool,
        ):
            # ---- one-time constants/weights ----
            wq = [
                cpool.tile([128, 3 * C], BF, tag=f"wq{i}", name=f"wq{i}")
                for i in range(6)
            ]
            wp = [
                cpool.tile([128, C], FR, tag=f"wp{i}", name=f"wp{i}") for i in range(6)
            ]
            for i in range(6):
                nc.sync.dma_start(wq[i][:], wqkT_d[128 * i : 128 * (i + 1), :])
                wps = wpool.tile([128, C], F32, tag="wps", name="wps")
                nc.sync.dma_start(wps[:], wpjT_d[128 * i : 128 * (i + 1), :])
                nc.vector.tensor_copy(wp[i][:], wps[:])
            sb_bqk = cpool.tile([1, 3 * C], BF, tag="bqk")
            nc.sync.dma_start(sb_bqk[:], bqk_d[:])
            ones = cpool.tile([1, 512], BF, tag="ones")
            nc.gpsimd.memset(ones[:], 1.0)
            ones_r = cpool.tile([1, 512], FR, tag="ones_r")
            onesf = wpool.tile([1, 512], F32, tag="onesf", name="onesf")
            nc.gpsimd.memset(onesf[:], 1.0)
            nc.vector.tensor_copy(ones_r[:], onesf[:])
            bpj_r = cpool.tile([1, C], FR, tag="bpj_r")
            bpjf = wpool.tile([1, C], F32, tag="bpjf", name="bpjf")
            nc.sync.dma_start(bpjf[:], bpj_d[:])
            nc.vector.tensor_copy(bpj_r[:], bpjf[:])
            ident = cpool.tile([128, 128], BF, tag="ident")
            make_identity(nc, ident[:])

            for b in range(NB):
                # ---- load xT ----
                xT = [
                    wpool.tile([128, N], BF, tag=f"xT{i}", name=f"xT{i}", bufs=1)
                    for i in range(6)
                ]
                for i in range(6):
                    eng = nc.scalar if b == 0 else nc.sync
                    eng.dma_start(xT[i][:], xT_d[b, 128 * i : 128 * (i + 1), :])

                # ---- phase Q: qkT c-major (q,k rows), bias via K=1 matmul ----
                qkT = [None] * 12
                for m in [0, 6, 1, 7, 2, 8, 3, 9, 4, 10, 5, 11]:
                    ps = ppool.tile([128, 2 * 512], F32, tag="A", name="psA")
                    ps2 = ppool.tile([128, 512], F32, tag="P", name="psP", bufs=4)
                    for j, (n0, nsz) in enumerate(NCHUNK):
                        dst = ps[:, 512 * j : 512 * j + nsz] if j < 2 else ps2[:, 0:nsz]
                        for kt in range(6):
                            nc.tensor.matmul(
                                dst,
                                wq[kt][:, 128 * m : 128 * (m + 1)],
                                xT[kt][:, n0 : n0 + nsz],
                                start=(kt == 0),
                                stop=False,
                            )
                        nc.tensor.matmul(
                            dst,
                            sb_bqk[0:1, 128 * m : 128 * (m + 1)],
                            ones[0:1, 0:nsz],
                            start=False,
                            stop=True,
                        )
                    t = hpool.tile([128, N], BF, tag=f"qkT{m}", name=f"qkT{m}")
                    qkT[m] = t
                    nc.scalar.copy(
                        t[:, 0:784].rearrange("p (c x) -> p c x", c=2),
                        ps[:].rearrange("p (c x) -> p c x", c=2)[:, :, 0:392],
                    )
                    nc.scalar.copy(t[:, 784:1176], ps2[:, 0:392])

                # ---- phase V: pos-major v_ext with ones cols ----
                v_ext = []
                for pt, (p0, psz) in enumerate(POS_T):
                    ps = ppool.tile([128, 2 * 512], F32, tag="A", name="psA")
                    for c0, csz in [(0, 512), (512, 256)]:
                        for kt in range(6):
                            nc.tensor.matmul(
                                ps[0:psz, c0 : c0 + csz],
                                xT[kt][:, p0 : p0 + psz],
                                wq[kt][:, 2 * C + c0 : 2 * C + c0 + csz],
                                start=(kt == 0),
                                stop=False,
                            )
                        nc.tensor.matmul(
                            ps[0:psz, c0 : c0 + csz],
                            ones[0:1, 0:psz],
                            sb_bqk[0:1, 2 * C + c0 : 2 * C + c0 + csz],
                            start=False,
                            stop=True,
                        )
                    vt = hpool.tile([128, H * 65], BF, tag=f"vx{pt}", name=f"vx{pt}")
                    v_ext.append(vt)
                    if b == 0:
                        # bufs=1 slot memory persists across batches; evac only
                        # writes the 64 v columns, so ones survive
                        nc.vector.memset(
                            vt[:].rearrange("p (h e) -> p h e", e=65)[:, :, 64:65], 1.0
                        )
                    nc.scalar.copy(
                        vt[0:psz].rearrange("p (h e) -> p h e", e=65)[:, :, 0:64],
                        ps[0:psz, 0:768].rearrange("p (h d) -> p h d", d=64),
                    )

                # ---- agent pooling: sum 4x4 blocks of q_s -> agT (c-major) ----
                agT = []
                for ct in range(6):
                    t1 = wpool.tile([128, 196], F32, tag="t1")
                    qs = qkT[ct][:, N_MT:N]  # [128, 784], idx = i*28 + aj*4 + dj
                    q4 = qs.rearrange("p (x dj) -> p x dj", dj=4)
                    nc.gpsimd.tensor_add(t1[:, 0:196], q4[:, :, 0:1], q4[:, :, 1:2])
                    nc.gpsimd.tensor_add(t1[:, 0:196], t1[:, 0:196], q4[:, :, 2:3])
                    nc.gpsimd.tensor_add(t1[:, 0:196], t1[:, 0:196], q4[:, :, 3:4])
                    ag = wpool.tile([128, A], BF, tag=f"ag{ct}", name=f"ag{ct}")
                    agT.append(ag)
                    # t1 idx = 28*ai + 7*di + aj -> view (ai, aj, di)
                    t4 = t1[:, 0:196].rearrange("p (ai di aj) -> p ai aj di", ai=7, di=4)
                    t2 = wpool.tile([128, A], F32, tag="t2")
                    nc.gpsimd.tensor_add(t2[:, 0:A], t4[:, :, :, 0:1], t4[:, :, :, 1:2])
                    nc.gpsimd.tensor_add(t2[:, 0:A], t2[:, 0:A], t4[:, :, :, 2:3])
                    nc.gpsimd.tensor_add(ag[:, 0:A], t2[:, 0:A], t4[:, :, :, 3:4])

                # attention output accumulates here (c-major, fp32r)
                aoT = [
                    hpool.tile([128, N], FR, tag=f"ao{i}", name=f"ao{i}")
                    for i in range(6)
                ]

                # ---- stage 1: one head ----
                def stage1(h):
                    qt, qo = h // 2, (h % 2) * 64
                    scA = ppool.tile([128, 2 * 512], F32, tag="A", name="psA")
                    sc2 = ppool.tile([128, 512], F32, tag="P", name="psP", bufs=4)
                    scP = ppool.tile([128, 512], F32, tag="P", name="psP", bufs=4)
                    for j, (k0, ksz) in enumerate(KEY1_T):
                        dst = (
                            scA[0:ksz, 512 * j : 512 * j + 392]
                            if j < 2
                            else (sc2[0:ksz, 0:392] if j == 2 else scP[0:ksz, 0:392])
                        )
                        nc.tensor.matmul(
                            dst,
                            qkT[6 + qt][qo : qo + 64, k0 : k0 + ksz],
                            qkT[qt][qo : qo + 64, 0:N_MT],
                            start=True,
                            stop=True,
                        )
                    e1 = [
                        wpool.tile([128, 392], BF, tag="e1", name="e1", bufs=8)
                        for _ in range(4)
                    ]
                    nc.scalar.activation(
                        e1[0][:, 0:392], scA[:, 0:392], AF.Exp, scale=SCALE1
                    )
                    nc.scalar.activation(
                        e1[1][:, 0:392], scA[:, 512:904], AF.Exp, scale=SCALE1
                    )
                    nc.scalar.activation(
                        e1[2][:, 0:392], sc2[:, 0:392], AF.Exp, scale=SCALE1
                    )
                    nc.scalar.activation(
                        e1[3][0:8, 0:392], scP[0:8, 0:392], AF.Exp, scale=SCALE1
                    )
                    pv = ppool.tile([128, 512], F32, tag="P", name="psP", bufs=4)
                    for j, (k0, ksz) in enumerate(KEY1_T):
                        nc.tensor.matmul(
                            pv[0:65, 0:392],
                            v_ext[j][0:ksz, 65 * h : 65 * h + 65],
                            e1[j][0:ksz, 0:392],
                            start=(j == 0),
                            stop=(j == 3),
                        )
                    se = wpool.tile([1, 392], F32, tag="se")
                    nc.vector.tensor_copy(se[:, 0:392], pv[64:65, 0:392])
                    rc = wpool.tile([1, 392], F32, tag="rc")
                    nc.vector.reciprocal_approx_fast(out=rc[:, 0:392], in_=se[:, 0:392])
                    bc = wpool.tile([64, 392], F32, tag="bc")
                    nc.gpsimd.partition_broadcast(bc[:], rc[0:1, 0:392])
                    nc.vector.tensor_mul(
                        aoT[qt][qo : qo + 64, 0:N_MT], pv[0:64, 0:392], bc[:]
                    )

                # ---- stages 1+2+3 interleaved per head pair ----
                for p2 in range(6):
                    stage1(2 * p2)
                    stage1(2 * p2 + 1)
                    # stage 2 scores (both heads packed at partition 0/64)
                    scA = ppool.tile([128, 2 * 512], F32, tag="A", name="psA")
                    sc2 = ppool.tile([128, 512], F32, tag="P", name="psP", bufs=4)
                    for hp in range(2):
                        h = 2 * p2 + hp
                        qt, qo = h // 2, (h % 2) * 64
                        for j, (n0, nsz) in enumerate(NCHUNK):
                            dst = (
                                scA[64 * hp : 64 * hp + 49, 512 * j : 512 * j + nsz]
                                if j < 2
                                else sc2[64 * hp : 64 * hp + 49, 0:nsz]
                            )
                            nc.tensor.matmul(
                                dst,
                                agT[qt][qo : qo + 64, 0:A],
                                qkT[6 + qt][qo : qo + 64, n0 : n0 + nsz],
                                start=True,
                                stop=True,
                            )
                    e2 = wpool.tile([128, N], BF, tag="e2")
                    nc.scalar.activation(
                        e2[0:113, 0:784].rearrange("p (c x) -> p c x", c=2),
                        scA[0:113].rearrange("p (c x) -> p c x", c=2)[:, :, 0:392],
                        AF.Exp,
                        scale=SCALE23,
                    )
                    nc.scalar.activation(
                        e2[0:113, 784:1176], sc2[0:113, 0:392], AF.Exp, scale=SCALE23
                    )
                    av = wpool.tile([128, 65], BF, tag="avx")
                    nc.vector.memset(av[:, 64:65], 1.0)
                    # transposes: both heads at once ([113, ktsz] -> [ktsz, 113])
                    e2T = []
                    for half in range(2):
                        trp = ppool.tile([128, 5 * TSP], BF, tag="P", name="psTr", bufs=4)
                        for kk in range(5):
                            kt = 5 * half + kk
                            p0, psz = POS_T[kt]
                            nc.tensor.transpose(
                                trp[0:psz, TSP * kk : TSP * kk + 113],
                                e2[0:113, p0 : p0 + psz],
                                ident[0:113, 0:113],
                            )
                        eT = wpool.tile([128, 5 * TSP], BF, tag="e2T")
                        e2T.append(eT)
                        nc.vector.tensor_copy(eT[:, 0 : 5 * TSP], trp[:, 0 : 5 * TSP])
                    pv2 = ppool.tile([128, 512], F32, tag="P", name="psP", bufs=4)
                    for hp in range(2):
                        h = 2 * p2 + hp
                        o = 64 * hp
                        for kt, (p0, psz) in enumerate(POS_T):
                            eT = e2T[kt // 5]
                            cof = TSP * (kt % 5) + 64 * hp
                            nc.tensor.matmul(
                                pv2[o : o + 49, 0:65],
                                eT[0:psz, cof : cof + 49],
                                v_ext[kt][0:psz, 65 * h : 65 * h + 65],
                                start=(kt == 0),
                                stop=(kt == 9),
                            )
                    avr = wpool.tile([128, 1], F32, tag="avr")
                    nc.vector.reciprocal(avr[0:113, 0:1], pv2[0:113, 64:65])
                    nc.vector.tensor_scalar_mul(
                        av[0:113, 0:64], pv2[0:113, 0:64], avr[0:113, 0:1]
                    )

                    # ---- stage 3 for this pair ----
                    scB = ppool.tile([128, 2 * 512], F32, tag="A", name="psA")
                    for hp in range(2):
                        h = 2 * p2 + hp
                        qt, qo = h // 2, (h % 2) * 64
                        for cc in range(2):
                            nc.tensor.matmul(
                                scB[64 * hp : 64 * hp + 49, 512 * cc : 512 * cc + 392],
                                agT[qt][qo : qo + 64, 0:A],
                                qkT[qt][
                                    qo : qo + 64, N_MT + 392 * cc : N_MT + 392 * (cc + 1)
                                ],
                                start=True,
                                stop=True,
                            )
                    e3 = [
                        wpool.tile([128, 392], BF, tag="e3", name="e3", bufs=4)
                        for _ in range(2)
                    ]
                    nc.scalar.activation(
                        e3[0][0:113, 0:392], scB[0:113, 0:392], AF.Exp, scale=SCALE23
                    )
                    nc.scalar.activation(
                        e3[1][0:113, 0:392], scB[0:113, 512:904], AF.Exp, scale=SCALE23
                    )
                    for hp in range(2):
                        h = 2 * p2 + hp
                        qt, qo = h // 2, (h % 2) * 64
                        for cc in range(2):
                            pv = ppool.tile([128, 512], F32, tag="P", name="psP", bufs=4)
                            nc.tensor.matmul(
                                pv[0:65, 0:392],
                                av[64 * hp : 64 * hp + 49, 0:65],
                                e3[cc][64 * hp : 64 * hp + 49, 0:392],
                                start=True,
                                stop=True,
                            )
                            se = wpool.tile([1, 392], F32, tag="se")
                            nc.vector.tensor_copy(se[:, 0:392], pv[64:65, 0:392])
                            rc = wpool.tile([1, 392], F32, tag="rc")
                            nc.vector.reciprocal_approx_fast(
                                out=rc[:, 0:392], in_=se[:, 0:392]
                            )
                            bc = wpool.tile([64, 392], F32, tag="bc")
                            nc.gpsimd.partition_broadcast(bc[:], rc[0:1, 0:392])
                            nc.vector.tensor_mul(
                                aoT[qt][
                                    qo : qo + 64, N_MT + 392 * cc : N_MT + 392 * (cc + 1)
                                ],
                                pv[0:64, 0:392],
                                bc[:],
                            )

                # ---- proj: out[pos, c] = aoT.T @ wpjT + bias ----
                for pt, (p0, psz) in enumerate(POS_T):
                    ps = ppool.tile([128, 2 * 512], F32, tag="A", name="psA")
                    for c0, csz in [(0, 512), (512, 256)]:
                        for kt in range(6):
                            nc.tensor.matmul(
                                ps[0:psz, c0 : c0 + csz],
                                aoT[kt][:, p0 : p0 + psz],
                                wp[kt][:, c0 : c0 + csz],
                                start=(kt == 0),
                                stop=False,
                            )
                        nc.tensor.matmul(
                            ps[0:psz, c0 : c0 + csz],
                            ones_r[0:1, 0:psz],
                            bpj_r[0:1, c0 : c0 + csz],
                            start=False,
                            stop=True,
                        )
                    ob = wpool.tile([128, C], F32, tag="osb")
                    nc.vector.tensor_copy(ob[0:psz, :], ps[0:psz, 0:C])
                    nc.sync.dma_start(out_d[b, p0 : p0 + psz, :], ob[0:psz, :])

    nc.compile()
    return nc


_PROGRAM = None


def _get_program():
    global _PROGRAM
    if _PROGRAM is None:
        _PROGRAM = build_program()
    return _PROGRAM


def kernel(x, qkv_w, qkv_b, proj_w, proj_b, t_h=14, t_w=14, s_h=28, s_w=28, **kw):
    x = np.asarray(x, dtype=np.float32)
    bf = ml_dtypes.bfloat16
    xT = np.ascontiguousarray(x.transpose(0, 2, 1)).astype(bf)  # [B, C, N]
    wqkT = np.ascontiguousarray(np.asarray(qkv_w, dtype=np.float32).T).astype(bf)
    wpjT = np.ascontiguousarray(np.asarray(proj_w, dtype=np.float32).T)
    bqk = np.asarray(qkv_b, dtype=np.float32).reshape(1, -1).astype(bf)
    bpj = np.asarray(proj_b, dtype=np.float32).reshape(1, -1)

    nc = _get_program()
    in_maps = []
    for c in range(N_CORES):
        in_maps.append(
            {
                "xT": np.ascontiguousarray(xT[c * NB : (c + 1) * NB]),
                "wqkT": wqkT,
                "wpjT": wpjT,
                "bqk": bqk,
                "bpj": bpj,
            }
        )
    res = bass_utils.run_bass_kernel_spmd(nc, in_maps, core_ids=list(range(N_CORES)))
    out = np.concatenate([res.results[c]["out"] for c in range(N_CORES)], axis=0)
    return out.astype(np.float32)


if __name__ == "__main__":
    build_program()
    print("program built OK")


# revision 36
# speedup vs baseline: 1.4478x; 1.1581x over previous
"""AgentAttention Trainium2 kernel: 8-core data-parallel over batch.

Layouts (per core, 4 batches):
  xT      [4, 768, 1176] bf16  (c-major x)
  qkT     c-major q,k: 12 sbuf tiles [128, 1176] (tiles 0-5 = q rows, 6-11 = k rows)
  v_ext   pos-major v with per-head ones column (col 64): 10 tiles [128, 12*65]
  agT     pooled agent tokens (sums over 4x4 blocks), c-major [128, 49] x6
  aoT     c-major attention output (fp32r) [128, 1176] x6 -> proj -> pos-major out
Matmuls bf16 (fp32r for proj), fp32 psum. Softmax scale folded into ACT exp
scale (0.125 stage1; 0.125/16 stages 2/3 -- agent tokens are pooled SUMS).
Biases are added via K=1 matmul accumulation into psum.
PSUM: tag A = [128,1024] (2 banks) x2, tag P = [128,512] (1 bank) x4.
"""

import sys

sys.path.insert(0, "/opt/trn_rl_repo")

import numpy as np
import ml_dtypes

import concourse.bass as bass
import concourse.mybir as mybir
import concourse.tile as tile
from concourse import bacc, bass_utils
from concourse.masks import make_identity

BF = mybir.dt.bfloat16
F32 = mybir.dt.float32
FR = mybir.dt.float32r
AF = mybir.ActivationFunctionType
AX = mybir.AxisListType

N_CORES = 8
B, N, C = 32, 1176, 768
NB = B // N_CORES
H, HD = 12, 64
N_MT, N_S = 392, 784
A = 49
SCALE1 = 0.125
SCALE23 = 0.125 / 16.0

POS_T = [(pt * 128, min(128, N - pt * 128)) for pt in range(10)]
KEY1_T = [(0, 128), (128, 128), (256, 128), (384, 8)]
NCHUNK = [(0, 392), (392, 392), (784, 392)]
TSP = 116  # transpose chunk col spacing (>=113, even)


def build_program():
    nc = bacc.Bacc("TRN2", debug=False, num_devices=N_CORES)

    xT_d = nc.dram_tensor("xT", [NB, C, N], BF, kind="ExternalInput").ap()
    wqkT_d = nc.dram_tensor("wqkT", [C, 3 * C], BF, kind="ExternalInput").ap()
    wpjT_d = nc.dram_tensor("wpjT", [C, C], F32, kind="ExternalInput").ap()
    bqk_d = nc.dram_tensor("bqk", [1, 3 * C], BF, kind="ExternalInput").ap()
    bqkp_d = nc.dram_tensor("bqkp", [128, 12], F32, kind="ExternalInput").ap()
    bpj_d = nc.dram_tensor("bpj", [1, C], F32, kind="ExternalInput").ap()
    out_d = nc.dram_tensor("out", [NB, N, C], F32, kind="ExternalOutput").ap()

    with tile.TileContext(nc) as tc, nc.allow_low_precision(reason="attn bf16"):
        with (
            tc.tile_pool(name="const", bufs=1) as cpool,
            tc.tile_pool(name="work", bufs=2) as wpool,
            tc.tile_pool(name="hold", bufs=1) as hpool,
            tc.tile_pool(name="psum", bufs=2, space="PSUM") as ppool,
        ):
            # ---- one-time constants/weights ----
            wq = [
                cpool.tile([128, 3 * C], BF, tag=f"wq{i}", name=f"wq{i}")
                for i in range(6)
            ]
            wp = [
                cpool.tile([128, C], FR, tag=f"wp{i}", name=f"wp{i}") for i in range(6)
            ]
            for i in range(6):
                nc.sync.dma_start(wq[i][:], wqkT_d[128 * i : 128 * (i + 1), :])
                wps = wpool.tile([128, C], F32, tag="wps", name="wps")
                nc.sync.dma_start(wps[:], wpjT_d[128 * i : 128 * (i + 1), :])
                nc.vector.tensor_copy(wp[i][:], wps[:])
            sb_bqk = cpool.tile([1, 3 * C], BF, tag="bqk")
            nc.sync.dma_start(sb_bqk[:], bqk_d[:])
            bqkp = cpool.tile([128, 12], F32, tag="bqkp")
            nc.sync.dma_start(bqkp[:], bqkp_d[:])
            vb_bc = cpool.tile([128, C], BF, tag="vb_bc")
            nc.gpsimd.partition_broadcast(vb_bc[:], sb_bqk[0:1, 2 * C : 3 * C])
            bpjf = wpool.tile([1, C], F32, tag="bpjf", name="bpjf")
            nc.sync.dma_start(bpjf[:], bpj_d[:])
            pb_bc = cpool.tile([128, C], F32, tag="pb_bc")
            nc.gpsimd.partition_broadcast(pb_bc[:], bpjf[0:1, :])
            ident = cpool.tile([128, 128], BF, tag="ident")
            make_identity(nc, ident[:])

            for b in range(NB):
                # ---- load xT ----
                xT = [
                    wpool.tile([128, N], BF, tag=f"xT{i}", name=f"xT{i}", bufs=1)
                    for i in range(6)
                ]
                for i in range(6):
                    eng = nc.scalar if b == 0 else nc.sync
                    eng.dma_start(xT[i][:], xT_d[b, 128 * i : 128 * (i + 1), :])

                # ---- phase Q: qkT c-major (q,k rows), bias via K=1 matmul ----
                qkT = [None] * 12
                for m in [0, 6, 1, 7, 2, 8, 3, 9, 4, 10, 5, 11]:
                    ps = ppool.tile([128, 2 * 512], F32, tag="A", name="psA")
                    ps2 = ppool.tile([128, 512], F32, tag="P", name="psP", bufs=4)
                    for j, (n0, nsz) in enumerate(NCHUNK):
                        dst = ps[:, 512 * j : 512 * j + nsz] if j < 2 else ps2[:, 0:nsz]
                        for kt in range(6):
                            nc.tensor.matmul(
                                dst,
                                wq[kt][:, 128 * m : 128 * (m + 1)],
                                xT[kt][:, n0 : n0 + nsz],
                                start=(kt == 0),
                                stop=(kt == 5),
                            )
                    t = hpool.tile([128, N], BF, tag=f"qkT{m}", name=f"qkT{m}")
                    qkT[m] = t
                    nc.vector.tensor_scalar_add(
                        t[:, 0:784].rearrange("p (c x) -> p c x", c=2),
                        ps[:].rearrange("p (c x) -> p c x", c=2)[:, :, 0:392],
                        bqkp[:, m : m + 1],
                    )
                    nc.vector.tensor_scalar_add(
                        t[:, 784:1176], ps2[:, 0:392], bqkp[:, m : m + 1]
                    )

                # ---- phase V: pos-major v_ext with ones cols ----
                v_ext = []
                for pt, (p0, psz) in enumerate(POS_T):
                    ps = ppool.tile([128, 2 * 512], F32, tag="A", name="psA")
                    for c0, csz in [(0, 512), (512, 256)]:
                        for kt in range(6):
                            nc.tensor.matmul(
                                ps[0:psz, c0 : c0 + csz],
                                xT[kt][:, p0 : p0 + psz],
                                wq[kt][:, 2 * C + c0 : 2 * C + c0 + csz],
                                start=(kt == 0),
                                stop=(kt == 5),
                            )
                    vt = hpool.tile([128, H * 65], BF, tag=f"vx{pt}", name=f"vx{pt}")
                    v_ext.append(vt)
                    if b == 0:
                        # bufs=1 slot memory persists across batches; evac only
                        # writes the 64 v columns, so ones survive
                        nc.vector.memset(
                            vt[:].rearrange("p (h e) -> p h e", e=65)[:, :, 64:65], 1.0
                        )
                    nc.vector.tensor_add(
                        vt[0:psz].rearrange("p (h e) -> p h e", e=65)[:, :, 0:64],
                        ps[0:psz, 0:768].rearrange("p (h d) -> p h d", d=64),
                        vb_bc[0:psz].rearrange("p (h d) -> p h d", d=64),
                    )

                # ---- agent pooling: sum 4x4 blocks of q_s -> agT (c-major) ----
                agT = []
                for ct in range(6):
                    t1 = wpool.tile([128, 196], F32, tag="t1")
                    qs = qkT[ct][:, N_MT:N]  # [128, 784], idx = i*28 + aj*4 + dj
                    q4 = qs.rearrange("p (x dj) -> p x dj", dj=4)
                    nc.gpsimd.tensor_add(t1[:, 0:196], q4[:, :, 0:1], q4[:, :, 1:2])
                    nc.gpsimd.tensor_add(t1[:, 0:196], t1[:, 0:196], q4[:, :, 2:3])
                    nc.gpsimd.tensor_add(t1[:, 0:196], t1[:, 0:196], q4[:, :, 3:4])
                    ag = wpool.tile([128, A], BF, tag=f"ag{ct}", name=f"ag{ct}")
                    agT.append(ag)
                    # t1 idx = 28*ai + 7*di + aj -> view (ai, aj, di)
                    t4 = t1[:, 0:196].rearrange("p (ai di aj) -> p ai aj di", ai=7, di=4)
                    t2 = wpool.tile([128, A], F32, tag="t2")
                    nc.gpsimd.tensor_add(t2[:, 0:A], t4[:, :, :, 0:1], t4[:, :, :, 1:2])
                    nc.gpsimd.tensor_add(t2[:, 0:A], t2[:, 0:A], t4[:, :, :, 2:3])
                    nc.gpsimd.tensor_add(ag[:, 0:A], t2[:, 0:A], t4[:, :, :, 3:4])

                # attention output accumulates here (c-major, fp32r)
                aoT = [
                    hpool.tile([128, N], FR, tag=f"ao{i}", name=f"ao{i}")
                    for i in range(6)
                ]

                # ---- stage 1: one head ----
                def stage1(h):
                    qt, qo = h // 2, (h % 2) * 64
                    scA = ppool.tile([128, 2 * 512], F32, tag="A", name="psA")
                    sc2 = ppool.tile([128, 512], F32, tag="P", name="psP", bufs=4)
                    scP = ppool.tile([128, 512], F32, tag="P", name="psP", bufs=4)
                    for j, (k0, ksz) in enumerate(KEY1_T):
                        dst = (
                            scA[0:ksz, 512 * j : 512 * j + 392]
                            if j < 2
                            else (sc2[0:ksz, 0:392] if j == 2 else scP[0:ksz, 0:392])
                        )
                        nc.tensor.matmul(
                            dst,
                            qkT[6 + qt][qo : qo + 64, k0 : k0 + ksz],
                            qkT[qt][qo : qo + 64, 0:N_MT],
                            start=True,
                            stop=True,
                        )
                    e1 = [
                        wpool.tile([128, 392], BF, tag="e1", name="e1", bufs=8)
                        for _ in range(4)
                    ]
                    nc.scalar.activation(
                        e1[0][:, 0:392], scA[:, 0:392], AF.Exp, scale=SCALE1
                    )
                    nc.scalar.activation(
                        e1[1][:, 0:392], scA[:, 512:904], AF.Exp, scale=SCALE1
                    )
                    nc.scalar.activation(
                        e1[2][:, 0:392], sc2[:, 0:392], AF.Exp, scale=SCALE1
                    )
                    nc.scalar.activation(
                        e1[3][0:8, 0:392], scP[0:8, 0:392], AF.Exp, scale=SCALE1
                    )
                    pv = ppool.tile([128, 512], F32, tag="P", name="psP", bufs=4)
                    for j, (k0, ksz) in enumerate(KEY1_T):
                        nc.tensor.matmul(
                            pv[0:65, 0:392],
                            v_ext[j][0:ksz, 65 * h : 65 * h + 65],
                            e1[j][0:ksz, 0:392],
                            start=(j == 0),
                            stop=(j == 3),
                        )
                    se = wpool.tile([1, 392], F32, tag="se")
                    nc.vector.tensor_copy(se[:, 0:392], pv[64:65, 0:392])
                    rc = wpool.tile([1, 392], F32, tag="rc")
                    nc.vector.reciprocal_approx_fast(out=rc[:, 0:392], in_=se[:, 0:392])
                    bc = wpool.tile([64, 392], F32, tag="bc")
                    nc.gpsimd.partition_broadcast(bc[:], rc[0:1, 0:392])
                    nc.vector.tensor_mul(
                        aoT[qt][qo : qo + 64, 0:N_MT], pv[0:64, 0:392], bc[:]
                    )

                # ---- stages 1+2+3 interleaved per head pair ----
                for p2 in range(6):
                    stage1(2 * p2)
                    stage1(2 * p2 + 1)
                    # stage 2 scores (both heads packed at partition 0/64)
                    scA = ppool.tile([128, 2 * 512], F32, tag="A", name="psA")
                    sc2 = ppool.tile([128, 512], F32, tag="P", name="psP", bufs=4)
                    for hp in range(2):
                        h = 2 * p2 + hp
                        qt, qo = h // 2, (h % 2) * 64
                        for j, (n0, nsz) in enumerate(NCHUNK):
                            dst = (
                                scA[64 * hp : 64 * hp + 49, 512 * j : 512 * j + nsz]
                                if j < 2
                                else sc2[64 * hp : 64 * hp + 49, 0:nsz]
                            )
                            nc.tensor.matmul(
                                dst,
                                agT[qt][qo : qo + 64, 0:A],
                                qkT[6 + qt][qo : qo + 64, n0 : n0 + nsz],
                                start=True,
                                stop=True,
                            )
                    e2 = wpool.tile([128, N], BF, tag="e2")
                    nc.scalar.activation(
                        e2[0:113, 0:784].rearrange("p (c x) -> p c x", c=2),
                        scA[0:113].rearrange("p (c x) -> p c x", c=2)[:, :, 0:392],
                        AF.Exp,
                        scale=SCALE23,
                    )
                    nc.scalar.activation(
                        e2[0:113, 784:1176], sc2[0:113, 0:392], AF.Exp, scale=SCALE23
                    )
                    av = wpool.tile([128, 65], BF, tag="avx")
                    nc.vector.memset(av[:, 64:65], 1.0)
                    # transposes: both heads at once ([113, ktsz] -> [ktsz, 113])
                    e2T = []
                    for half in range(2):
                        trp = ppool.tile([128, 5 * TSP], BF, tag="P", name="psTr", bufs=4)
                        for kk in range(5):
                            kt = 5 * half + kk
                            p0, psz = POS_T[kt]
                            nc.tensor.transpose(
                                trp[0:psz, TSP * kk : TSP * kk + 113],
                                e2[0:113, p0 : p0 + psz],
                                ident[0:113, 0:113],
                            )
                        eT = wpool.tile([128, 5 * TSP], BF, tag="e2T")
                        e2T.append(eT)
                        nc.vector.tensor_copy(eT[:, 0 : 5 * TSP], trp[:, 0 : 5 * TSP])
                    pv2 = ppool.tile([128, 512], F32, tag="P", name="psP", bufs=4)
                    for hp in range(2):
                        h = 2 * p2 + hp
                        o = 64 * hp
                        for kt, (p0, psz) in enumerate(POS_T):
                            eT = e2T[kt // 5]
                            cof = TSP * (kt % 5) + 64 * hp
                            nc.tensor.matmul(
                                pv2[o : o + 49, 0:65],
                                eT[0:psz, cof : cof + 49],
                                v_ext[kt][0:psz, 65 * h : 65 * h + 65],
                                start=(kt == 0),
                                stop=(kt == 9),
                            )
                    avr = wpool.tile([128, 1], F32, tag="avr")
                    nc.vector.reciprocal(avr[0:113, 0:1], pv2[0:113, 64:65])
                    nc.vector.tensor_scalar_mul(
                        av[0:113, 0:64], pv2[0:113, 0:64], avr[0:113, 0:1]
                    )

                    # ---- stage 3 for this pair ----
                    scB = ppool.tile([128, 2 * 512], F32, tag="A", name="psA")
                    for hp in range(2):
                        h = 2 * p2 + hp
                        qt, qo = h // 2, (h % 2) * 64
                        for cc in range(2):
                            nc.tensor.matmul(
                                scB[64 * hp : 64 * hp + 49, 512 * cc : 512 * cc + 392],
                                agT[qt][qo : qo + 64, 0:A],
                                qkT[qt][
                                    qo : qo + 64, N_MT + 392 * cc : N_MT + 392 * (cc + 1)
                                ],
                                start=True,
                                stop=True,
                            )
                    e3 = [
                        wpool.tile([128, 392], BF, tag="e3", name="e3", bufs=4)
                        for _ in range(2)
                    ]
                    nc.scalar.activation(
                        e3[0][0:113, 0:392], scB[0:113, 0:392], AF.Exp, scale=SCALE23
                    )
                    nc.scalar.activation(
                        e3[1][0:113, 0:392], scB[0:113, 512:904], AF.Exp, scale=SCALE23
                    )
                    for hp in range(2):
                        h = 2 * p2 + hp
                        qt, qo = h // 2, (h % 2) * 64
                        for cc in range(2):
                            pv = ppool.tile([128, 512], F32, tag="P", name="psP", bufs=4)
                            nc.tensor.matmul(
                                pv[0:65, 0:392],
                                av[64 * hp : 64 * hp + 49, 0:65],
                                e3[cc][64 * hp : 64 * hp + 49, 0:392],
                                start=True,
                                stop=True,
                            )
                            se = wpool.tile([1, 392], F32, tag="se")
                            nc.vector.tensor_copy(se[:, 0:392], pv[64:65, 0:392])
                            rc = wpool.tile([1, 392], F32, tag="rc")
                            nc.vector.reciprocal_approx_fast(
                                out=rc[:, 0:392], in_=se[:, 0:392]
                            )
                            bc = wpool.tile([64, 392], F32, tag="bc")
                            nc.gpsimd.partition_broadcast(bc[:], rc[0:1, 0:392])
                            nc.vector.tensor_mul(
                                aoT[qt][
                                    qo : qo + 64, N_MT + 392 * cc : N_MT + 392 * (cc + 1)
                                ],
                                pv[0:64, 0:392],
                                bc[:],
                            )

                # ---- proj: out[pos, c] = aoT.T @ wpjT + bias ----
                for pt, (p0, psz) in enumerate(POS_T):
                    ps = ppool.tile([128, 2 * 512], F32, tag="A", name="psA")
                    for c0, csz in [(0, 512), (512, 256)]:
                        for kt in range(6):
                            nc.tensor.matmul(
                                ps[0:psz, c0 : c0 + csz],
                                aoT[kt][:, p0 : p0 + psz],
                                wp[kt][:, c0 : c0 + csz],
                                start=(kt == 0),
                                stop=(kt == 5),
                            )
                    ob = wpool.tile([128, C], F32, tag="osb")
                    nc.vector.tensor_add(ob[0:psz, :], ps[0:psz, 0:C], pb_bc[0:psz, :])
                    nc.sync.dma_start(out_d[b, p0 : p0 + psz, :], ob[0:psz, :])

    nc.compile()
    return nc


_PROGRAM = None


def _get_program():
    global _PROGRAM
    if _PROGRAM is None:
        _PROGRAM = build_program()
    return _PROGRAM


def kernel(x, qkv_w, qkv_b, proj_w, proj_b, t_h=14, t_w=14, s_h=28, s_w=28, **kw):
    x = np.asarray(x, dtype=np.float32)
    bf = ml_dtypes.bfloat16
    xT = np.ascontiguousarray(x.transpose(0, 2, 1)).astype(bf)  # [B, C, N]
    wqkT = np.ascontiguousarray(np.asarray(qkv_w, dtype=np.float32).T).astype(bf)
    wpjT = np.ascontiguousarray(np.asarray(proj_w, dtype=np.float32).T)
    bqk = np.asarray(qkv_b, dtype=np.float32).reshape(1, -1).astype(bf)
    bqkp = np.ascontiguousarray(
        np.asarray(qkv_b, dtype=np.float32)[: 2 * 768].reshape(12, 128).T
    ).astype(np.float32)
    bpj = np.asarray(proj_b, dtype=np.float32).reshape(1, -1)

    nc = _get_program()
    in_maps = []
    for c in range(N_CORES):
        in_maps.append(
            {
                "xT": np.ascontiguousarray(xT[c * NB : (c + 1) * NB]),
                "wqkT": wqkT,
                "wpjT": wpjT,
                "bqk": bqk,
                "bqkp": bqkp,
                "bpj": bpj,
            }
        )
    res = bass_utils.run_bass_kernel_spmd(nc, in_maps, core_ids=list(range(N_CORES)))
    out = np.concatenate([res.results[c]["out"] for c in range(N_CORES)], axis=0)
    return out.astype(np.float32)


if __name__ == "__main__":
    build_program()
    print("program built OK")
